# revision 1
# baseline (speedup 1.0000x reference)
"""PoseMetrics (mpjpe / pa_mpjpe / accel_error) Trainium2 Bass kernel.

Full inputs: pred/target [524288, 3, 14] fp32. Output: [3] fp32.

Strategy (pure data parallel, 8 cores x 65536 samples):
  - batch-major layout: 128 partitions x 512 samples/partition, processed in
    8 chunks of 64 samples (innermost axis = samples -> fp16 2x DVE mode,
    with step-0 broadcasts on outer dims).
  - Kabsch/SVD replaced by a closed form: cross-covariance H per sample,
    Cardano eigenvalues of K = H^T H -> lambda_max of the Davenport quartic,
    then Markley's FOAM formula for the optimal rotation R (handles the
    det<0 reflection case via lambda = s1+s2+sign(det)*s3). All FOAM math is
    fp32; bulk per-joint slabs are fp16 (storage) with fp32 ALUs.
  - Each core returns [128, 24] partial sums (3 metrics x 8 chunk slots);
    the host reduces in float64 and divides by the element counts.
"""

import numpy as np

import concourse.bass as bass
import concourse.bacc as bacc
import concourse.mybir as mybir
import concourse.tile as tile
from concourse.bass_utils import run_bass_kernel_spmd

F32 = mybir.dt.float32
F16 = mybir.dt.float16
AX = mybir.AluOpType
AF = mybir.ActivationFunctionType

N_CORES = 8
B_FULL = 524288
B_LOC = B_FULL // N_CORES          # 65536
P = 128                            # partitions
S = B_LOC // P                     # 512 samples per partition
NB = 64                            # samples per chunk (per partition)
NCHUNK = S // NB                   # 8
CJ = 42                            # 3*14
PI = float(np.pi)
DEBUG = False
PHASES = 3  # 1=pass1 only, 2=+FOAM, 3=full


def _load_convert(nc, loadp, halfp, view, ci, name, stage=None):
    """DMA one fp32 chunk and produce the fp16 J-major tile [128,3,14,NB].

    If `stage` (DRAM [P, NCHUNK, 3*14*NB] f16) is given, also write the fp16
    tile out so pass 3 can re-read it without re-converting.
    """
    x32 = loadp.tile([P, NB, CJ], F32, tag=f"{name}32", name=f"{name}32")
    nc.sync.dma_start(x32[:], view[:, ci * NB:(ci + 1) * NB, :])
    x16 = halfp.tile([P, 3, 14, NB], F16, tag=f"{name}16", name=f"{name}16")
    # [p, s, (c j)] -> [p, c, j, s]  (strided read, contiguous fp16 write)
    src = x32[:].rearrange("p s (c j) -> p c j s", c=3, j=14)
    nc.scalar.copy(x16[:], src)
    if stage is not None:
        nc.sync.dma_start(
            stage[:, ci, :].rearrange("p (c j s) -> p c j s", c=3, j=14, s=NB),
            x16[:])
    return x16


def _load_staged(nc, halfp, stage, ci, name, bufs=None):
    x16 = halfp.tile([P, 3, 14, NB], F16, tag=f"{name}16", name=f"{name}16", bufs=bufs)
    nc.sync.dma_start(
        x16[:],
        stage[:, ci, :].rearrange("p (c j s) -> p c j s", c=3, j=14, s=NB))
    return x16


def _tree14(nc, workp, x, out, tag, eng=None):
    """Sum 14 J-slices of x [128, ..., 14, NB] (fp16) into out [..., 1, NB] fp32.

    Tree: 7+7 -> (3+3, keep 6) -> pairs; final add in fp32.
    """
    eng = eng or nc.vector
    pre = x.shape[1:-2]  # middle dims, e.g. (3,3) or (3,)
    l1 = workp.tile([P, *pre, 7, NB], F16, tag=f"tr{tag[0]}_l1", name=f"{tag}_l1", bufs=1)
    eng.tensor_tensor(l1[:], x[..., 0:7, :], x[..., 7:14, :], op=AX.add)
    l2 = workp.tile([P, *pre, 3, NB], F16, tag=f"tr{tag[0]}_l2", name=f"{tag}_l2", bufs=1)
    eng.tensor_tensor(l2[:], l1[..., 0:3, :], l1[..., 3:6, :], op=AX.add)
    l3 = workp.tile([P, *pre, 1, NB], F16, tag=f"tr{tag[0]}_l3", name=f"{tag}_l3", bufs=1)
    eng.tensor_tensor(l3[:], l2[..., 0:1, :], l2[..., 1:2, :], op=AX.add)
    l4 = workp.tile([P, *pre, 1, NB], F16, tag=f"tr{tag[0]}_l4", name=f"{tag}_l4", bufs=1)
    eng.tensor_tensor(l4[:], l3[:], l2[..., 2:3, :], op=AX.add)
    l5 = workp.tile([P, *pre, 1, NB], F16, tag=f"tr{tag[0]}_l5", name=f"{tag}_l5", bufs=1)
    eng.tensor_tensor(l5[:], l4[:], l1[..., 6:7, :], op=AX.add)
    eng.tensor_copy(out, l5[:])


def _sum3sq_sqrt_acc(nc, workp, d, nj, acc_slice, tag, eng=None):
    """d [128,3,nj,NB] fp16 -> sum_c d^2 -> sqrt -> accumulate into acc [128,1]."""
    eng = eng or nc.vector
    sq = workp.tile([P, 3, nj, NB], F16, tag="sq3", name=f"{tag}_sq", bufs=1)
    nc.scalar.square(sq[:], d[:])
    s1 = workp.tile([P, nj, NB], F16, tag="ssum1", name=f"{tag}_s1", bufs=1)
    eng.tensor_tensor(s1[:], sq[:, 0], sq[:, 1], op=AX.add)
    s2 = workp.tile([P, nj, NB], F16, tag="ssum2", name=f"{tag}_s2", bufs=1)
    eng.tensor_tensor(s2[:], s1[:], sq[:, 2], op=AX.add)
    scr = workp.tile([P, nj, NB], F16, tag="sqscr", name=f"{tag}_scr", bufs=1)
    nc.scalar.activation(scr[:], s2[:], AF.Sqrt, accum_out=acc_slice)




def _foam(nc, sp_, chp, G, SP, ST, R16, V16, b_pih, b_pih23, fo, SF, dbg=None):
    """FOAM rotation for samples [fo, fo+SF) of the per-partition range.

    Engine split: DVE does the slab math and the Newton chain; GPSIMD (Pool)
    does the fp32 small-channel cofactor chains (det3 x2, I2, adjH). The
    assembly slabs run in fp16 (2x DVE) with a 1/8 prescale folded into H16
    to keep fp16 in range.
    """
    fs = slice(fo, fo + SF)
    Gv = G[:, :, :, fs]
    SPv = SP[:, :, fs]
    STv = ST[:, :, fs]
    S3 = [P, 3, 3, SF]

    def slab(name):
        return sp_.tile(S3, F32, tag="slab", name=name)

    def slab16(name, tag=None, bufs=None):
        return sp_.tile(S3, F16, tag=tag or "slab16", name=name, bufs=bufs)

    def ch(name):
        return chp.tile([P, SF], F32, tag="ch", name=name)

    def named(tag):
        return chp.tile([P, SF], F32, tag=tag, name=tag, bufs=1)

    # H = G - SP ST^T / 14
    H = sp_.tile(S3, F32, tag="H", bufs=1)
    outer = slab("outer")
    nc.vector.tensor_tensor(
        outer[:], SPv.unsqueeze(2).broadcast_to(S3),
        STv.unsqueeze(1).broadcast_to(S3), op=AX.mult)
    nc.vector.scalar_tensor_tensor(
        H[:], outer[:], -1.0 / 14.0, Gv, op0=AX.mult, op1=AX.add)

    # K = H^T H via 3 outer products over c
    K = sp_.tile(S3, F32, tag="K", bufs=1)
    t0 = slab("t0")
    nc.vector.tensor_tensor(t0[:], H[:, 0].unsqueeze(2).broadcast_to(S3),
                            H[:, 0].unsqueeze(1).broadcast_to(S3), op=AX.mult)
    t1 = slab("t1")
    nc.vector.tensor_tensor(t1[:], H[:, 1].unsqueeze(2).broadcast_to(S3),
                            H[:, 1].unsqueeze(1).broadcast_to(S3), op=AX.mult)
    nc.vector.tensor_tensor(K[:], t0[:], t1[:], op=AX.add)
    t2 = slab("t2")
    nc.vector.tensor_tensor(t2[:], H[:, 2].unsqueeze(2).broadcast_to(S3),
                            H[:, 2].unsqueeze(1).broadcast_to(S3), op=AX.mult)
    nc.vector.tensor_tensor(K[:], K[:], t2[:], op=AX.add)

    m2 = named("m2")
    nc.vector.tensor_tensor(m2[:], K[:, 0, 0], K[:, 1, 1], op=AX.add)
    nc.vector.tensor_tensor(m2[:], m2[:], K[:, 2, 2], op=AX.add)

    def det3(eng, A, out_ch):
        c1 = ch("det_c1"); c2 = ch("det_c2"); acc = ch("det_acc")
        eng.tensor_tensor(c1[:], A[:, 1, 1], A[:, 2, 2], op=AX.mult)
        eng.tensor_tensor(c2[:], A[:, 1, 2], A[:, 2, 1], op=AX.mult)
        eng.tensor_tensor(c1[:], c1[:], c2[:], op=AX.subtract)
        eng.tensor_tensor(acc[:], A[:, 0, 0], c1[:], op=AX.mult)
        eng.tensor_tensor(c1[:], A[:, 1, 0], A[:, 2, 2], op=AX.mult)
        eng.tensor_tensor(c2[:], A[:, 1, 2], A[:, 2, 0], op=AX.mult)
        eng.tensor_tensor(c1[:], c1[:], c2[:], op=AX.subtract)
        eng.tensor_tensor(c1[:], A[:, 0, 1], c1[:], op=AX.mult)
        eng.tensor_tensor(acc[:], acc[:], c1[:], op=AX.subtract)
        eng.tensor_tensor(c1[:], A[:, 1, 0], A[:, 2, 1], op=AX.mult)
        eng.tensor_tensor(c2[:], A[:, 1, 1], A[:, 2, 0], op=AX.mult)
        eng.tensor_tensor(c1[:], c1[:], c2[:], op=AX.subtract)
        eng.tensor_tensor(c1[:], A[:, 0, 2], c1[:], op=AX.mult)
        eng.tensor_tensor(out_ch[:], acc[:], c1[:], op=AX.add)

    detH = named("detH")
    det3(nc.gpsimd, H, detH)

    # Cardano bound pieces (for the Newton start): q, p
    q = named("q")
    nc.scalar.mul(q[:], m2[:], 1.0 / 3.0)
    o01 = ch("o01"); o02 = ch("o02"); o12 = ch("o12")
    nc.scalar.square(o01[:], K[:, 0, 1])
    nc.scalar.square(o02[:], K[:, 0, 2])
    nc.scalar.square(o12[:], K[:, 1, 2])
    osum = ch("osum")
    nc.vector.tensor_tensor(osum[:], o01[:], o02[:], op=AX.add)
    nc.vector.tensor_tensor(osum[:], osum[:], o12[:], op=AX.add)
    dsum = ch("dsum"); kd = ch("kd"); kd2 = ch("kd2")
    nc.vector.tensor_tensor(kd[:], K[:, 0, 0], q[:], op=AX.subtract)
    nc.scalar.square(dsum[:], kd[:])
    nc.vector.tensor_tensor(kd[:], K[:, 1, 1], q[:], op=AX.subtract)
    nc.scalar.square(kd2[:], kd[:])
    nc.vector.tensor_tensor(dsum[:], dsum[:], kd2[:], op=AX.add)
    nc.vector.tensor_tensor(kd[:], K[:, 2, 2], q[:], op=AX.subtract)
    nc.scalar.square(kd2[:], kd[:])
    nc.vector.tensor_tensor(dsum[:], dsum[:], kd2[:], op=AX.add)
    p2 = named("p2")
    nc.vector.scalar_tensor_tensor(p2[:], osum[:], 2.0, dsum[:], op0=AX.mult, op1=AX.add)
    nc.vector.tensor_scalar_add(p2[:], p2[:], 1e-30)
    pC = named("pC")
    nc.scalar.activation(pC[:], p2[:], AF.Sqrt, scale=1.0 / 6.0)

    # I2 (on Pool), I3 = det K (on Pool)
    I2 = named("I2"); mm = ch("mm")
    nc.gpsimd.tensor_tensor(I2[:], K[:, 0, 0], K[:, 1, 1], op=AX.mult)
    nc.gpsimd.tensor_tensor(I2[:], I2[:], o01[:], op=AX.subtract)
    nc.gpsimd.tensor_tensor(mm[:], K[:, 0, 0], K[:, 2, 2], op=AX.mult)
    nc.gpsimd.tensor_tensor(mm[:], mm[:], o02[:], op=AX.subtract)
    nc.gpsimd.tensor_tensor(I2[:], I2[:], mm[:], op=AX.add)
    nc.gpsimd.tensor_tensor(mm[:], K[:, 1, 1], K[:, 2, 2], op=AX.mult)
    nc.gpsimd.tensor_tensor(mm[:], mm[:], o12[:], op=AX.subtract)
    nc.gpsimd.tensor_tensor(I2[:], I2[:], mm[:], op=AX.add)
    I3 = named("I3")
    det3(nc.gpsimd, K, I3)

    # adjH on Pool, fp16 output, prescaled by 1/8 (via aw1 * 0.125)
    adjH16 = sp_.tile(S3, F16, tag="adjH16", bufs=1)
    idx = [
        (0, 0, (1, 1), (2, 2), (1, 2), (2, 1)),
        (0, 1, (0, 2), (2, 1), (0, 1), (2, 2)),
        (0, 2, (0, 1), (1, 2), (0, 2), (1, 1)),
        (1, 0, (1, 2), (2, 0), (1, 0), (2, 2)),
        (1, 1, (0, 0), (2, 2), (0, 2), (2, 0)),
        (1, 2, (0, 2), (1, 0), (0, 0), (1, 2)),
        (2, 0, (1, 0), (2, 1), (1, 1), (2, 0)),
        (2, 1, (0, 1), (2, 0), (0, 0), (2, 1)),
        (2, 2, (0, 0), (1, 1), (0, 1), (1, 0)),
    ]
    aw1 = ch("aw1"); aw2 = ch("aw2")
    for (i, j, (a1, a2), (b1, b2), (c1_, c2_), (d1, d2)) in idx:
        nc.gpsimd.tensor_tensor(aw1[:], H[:, a1, a2], H[:, b1, b2], op=AX.mult)
        nc.gpsimd.tensor_tensor(aw2[:], H[:, c1_, c2_], H[:, d1, d2], op=AX.mult)
        nc.gpsimd.tensor_tensor(adjH16[:, i, j], aw1[:], aw2[:], op=AX.subtract)

    # Largest/smallest eigenvalues of K via Newton on the characteristic cubic
    # f(x) = x^3 - I1 x^2 + I2 x - I3 (I1 = m2).  mu1 from above (x0 = q + 2p,
    # the Cardano bound), mu3 from below (x0 = 0); stacked [P, 2, SF].
    # Trig-free: the ACT Sin/Arctan tables are too coarse for lambda.
    S2F = [P, 2, SF]
    X = chp.tile(S2F, F32, tag="X", name="X", bufs=1)
    nc.vector.scalar_tensor_tensor(X[:, 0], pC[:], 2.0, q[:], op0=AX.mult, op1=AX.add)
    nc.gpsimd.memset(X[:, 1], 0.0)
    I1b = m2[:].unsqueeze(1).broadcast_to(S2F)
    I2b = I2[:].unsqueeze(1).broadcast_to(S2F)
    I3b = I3[:].unsqueeze(1).broadcast_to(S2F)
    na = chp.tile(S2F, F32, tag="na", name="na", bufs=1)
    nb = chp.tile(S2F, F32, tag="nb", name="nb", bufs=1)
    for _ in range(4):
        nc.vector.tensor_tensor(na[:], X[:], I1b, op=AX.subtract)
        nc.vector.tensor_tensor(na[:], na[:], X[:], op=AX.mult)
        nc.vector.tensor_tensor(na[:], na[:], I2b, op=AX.add)
        nc.vector.tensor_tensor(na[:], na[:], X[:], op=AX.mult)
        nc.vector.tensor_tensor(na[:], na[:], I3b, op=AX.subtract)   # f
        nc.vector.tensor_scalar_mul(nb[:], X[:], 3.0)
        nc.vector.scalar_tensor_tensor(nb[:], I1b, -2.0, nb[:], op0=AX.mult, op1=AX.add)
        nc.vector.tensor_tensor(nb[:], nb[:], X[:], op=AX.mult)
        nc.vector.tensor_tensor(nb[:], nb[:], I2b, op=AX.add)        # f'
        nc.vector.reciprocal(nb[:], nb[:])
        nc.vector.tensor_tensor(na[:], na[:], nb[:], op=AX.mult)
        nc.vector.tensor_tensor(X[:], X[:], na[:], op=AX.subtract)

    mu1 = ch("mu1"); mu2 = ch("mu2"); mu3 = ch("mu3")
    nc.vector.tensor_scalar_max(mu1[:], X[:, 0], 0.0)
    nc.vector.tensor_scalar_max(mu3[:], X[:, 1], 0.0)
    nc.vector.tensor_tensor(mu2[:], mu1[:], mu3[:], op=AX.add)
    nc.vector.tensor_tensor(mu2[:], m2[:], mu2[:], op=AX.subtract)
    nc.vector.tensor_scalar_max(mu2[:], mu2[:], 0.0)

    s1 = ch("s1"); s2 = ch("s2"); s3 = ch("s3")
    for mu, s_ in ((mu1, s1), (mu2, s2), (mu3, s3)):
        nc.scalar.sqrt(s_[:], mu[:])
    sgn = ch("sgn")
    nc.scalar.sign(sgn[:], detH[:])
    lam = named("lam")
    nc.vector.tensor_tensor(lam[:], s1[:], s2[:], op=AX.add)
    nc.vector.tensor_tensor(s3[:], sgn[:], s3[:], op=AX.mult)
    nc.vector.tensor_tensor(lam[:], lam[:], s3[:], op=AX.add)

    # alpha2 = lam^2 + m2 ; zeta2 = (lam^2 - m2) lam - 2 det (floored)
    lam2 = ch("lam2"); alpha2 = named("alpha2")
    nc.scalar.square(lam2[:], lam[:])
    nc.vector.tensor_tensor(alpha2[:], lam2[:], m2[:], op=AX.add)
    zt = ch("zt")
    nc.vector.tensor_tensor(zt[:], lam2[:], m2[:], op=AX.subtract)
    nc.vector.tensor_tensor(zt[:], zt[:], lam[:], op=AX.mult)
    zeta2 = ch("zeta2")
    nc.vector.scalar_tensor_tensor(zeta2[:], detH[:], -2.0, zt[:], op0=AX.mult, op1=AX.add)
    m2s = ch("m2s"); zfl = ch("zfl")
    nc.scalar.sqrt(m2s[:], m2[:])
    nc.vector.tensor_tensor(zfl[:], m2[:], m2s[:], op=AX.mult)
    nc.vector.tensor_scalar_mul(zfl[:], zfl[:], 1e-4)
    nc.vector.tensor_tensor(zeta2[:], zeta2[:], zfl[:], op=AX.max)
    rz = named("rz")
    nc.vector.reciprocal(rz[:], zeta2[:])

    # fp16 prescaled copies for 2x assembly: H16 = H/8, K16 = K,
    # alpha2_16 = alpha2, lam2x16 = 2*lam/8
    H16 = slab16("H16", tag="H16", bufs=1)
    nc.vector.tensor_scalar_mul(H16[:], H[:], 0.125)
    K16 = slab16("K16", tag="K16", bufs=1)
    nc.vector.tensor_copy(K16[:], K[:])
    a2_16 = chp.tile([P, SF], F16, tag="a2_16", name="a2_16", bufs=1)
    nc.vector.tensor_copy(a2_16[:], alpha2[:])
    l2x16 = chp.tile([P, SF], F16, tag="l2x16", name="l2x16", bufs=1)
    nc.vector.tensor_scalar_mul(l2x16[:], lam[:], 0.25)

    # M3/8 = K (H/8)^T in fp16
    M38 = slab16("M38", tag="M38", bufs=1)
    u0 = slab16("u0")
    nc.vector.tensor_tensor(u0[:], K16[:, :, 0].unsqueeze(2).broadcast_to(S3),
                            H16[:, :, 0].unsqueeze(1).broadcast_to(S3), op=AX.mult)
    u1 = slab16("u1")
    nc.vector.tensor_tensor(u1[:], K16[:, :, 1].unsqueeze(2).broadcast_to(S3),
                            H16[:, :, 1].unsqueeze(1).broadcast_to(S3), op=AX.mult)
    nc.vector.tensor_tensor(M38[:], u0[:], u1[:], op=AX.add)
    u2 = slab16("u2")
    nc.vector.tensor_tensor(u2[:], K16[:, :, 2].unsqueeze(2).broadcast_to(S3),
                            H16[:, :, 2].unsqueeze(1).broadcast_to(S3), op=AX.mult)
    nc.vector.tensor_tensor(M38[:], M38[:], u2[:], op=AX.add)

    # num2/8 = alpha2*H^T/8 + (2 lam/8)*adjH - 2*M3/8   (all fp16, 2x)
    Ht16 = H16[:].transpose([0, 2, 1, 3])
    tB = slab16("tB")
    nc.vector.tensor_tensor(
        tB[:], a2_16[:].unsqueeze(1).unsqueeze(2).broadcast_to(S3), Ht16, op=AX.mult)
    vB = slab16("vB")
    nc.vector.tensor_tensor(
        vB[:], l2x16[:].unsqueeze(1).unsqueeze(2).broadcast_to(S3), adjH16[:], op=AX.mult)
    nc.vector.tensor_tensor(tB[:], tB[:], vB[:], op=AX.add)
    m3m2 = slab16("m3m2")
    nc.vector.tensor_scalar_mul(m3m2[:], M38[:], -2.0)
    num2 = slab16("num2")
    nc.vector.tensor_tensor(num2[:], tB[:], m3m2[:], op=AX.add)

    # R = (num2/8) * (8/zeta2), clamped
    rz8 = named("rz8")
    nc.vector.tensor_scalar_mul(rz8[:], rz[:], 8.0)
    R16v = R16[:, :, :, fs]
    nc.vector.tensor_tensor(
        R16v, num2[:], rz8[:].unsqueeze(1).unsqueeze(2).broadcast_to(S3), op=AX.mult)
    nc.vector.tensor_scalar(R16v, R16v, 4.0, -4.0, op0=AX.min, op1=AX.max)

    # V = (ST - R SP) / 14
    SP16 = chp.tile([P, 3, SF], F16, tag="SP16", name="SP16", bufs=1)
    nc.vector.tensor_copy(SP16[:], SPv)
    pv_ = slab16("pv_")
    nc.vector.tensor_tensor(pv_[:], R16v, SP16[:].unsqueeze(1).broadcast_to(S3), op=AX.mult)
    RS = chp.tile([P, 3, SF], F16, tag="RS", name="RS", bufs=1)
    nc.vector.tensor_tensor(RS[:], pv_[:, :, 0], pv_[:, :, 1], op=AX.add)
    nc.vector.tensor_tensor(RS[:], RS[:], pv_[:, :, 2], op=AX.add)
    RSf = chp.tile([P, 3, SF], F32, tag="RSf", name="RSf", bufs=1)
    nc.vector.tensor_tensor(RSf[:], STv, RS[:], op=AX.subtract)
    nc.vector.tensor_scalar_mul(V16[:, :, fs], RSf[:], 1.0 / 14.0)

    if dbg is not None:
        for i_, t_ in enumerate((m2, detH, pC, q, I2, lam, zeta2, rz)):
            nc.gpsimd.tensor_copy(dbg[:, i_, fs], t_[:])


def build_bass():
    nc = bacc.Bacc("TRN2")
    pred = nc.dram_tensor("pred", [B_LOC, CJ], F32, kind="ExternalInput")
    targ = nc.dram_tensor("target", [B_LOC, CJ], F32, kind="ExternalInput")
    out = nc.dram_tensor("out", [P, 24], F32, kind="ExternalOutput")
    pstage = nc.dram_tensor("pstage", [P, NCHUNK, 3 * 14 * NB], F16)
    tstage = nc.dram_tensor("tstage", [P, NCHUNK, 3 * 14 * NB], F16)
    if DEBUG:
        dbg_t = nc.dram_tensor("dbg", [P, 8 * S], F32, kind="ExternalOutput")
        dbgr_t = nc.dram_tensor("dbgr", [P, 9 * S], F32, kind="ExternalOutput")

    pv = pred[:].rearrange("(p n) d -> p n d", p=P)   # [128, 512, 42]
    tv = targ[:].rearrange("(p n) d -> p n d", p=P)

    with tile.TileContext(nc) as tc:
        with tc.tile_pool(name="persist", bufs=1) as pp:
            G = pp.tile([P, 3, 3, S], F32, tag="G")
            SP = pp.tile([P, 3, S], F32, tag="SP")
            ST = pp.tile([P, 3, S], F32, tag="ST")
            accM = pp.tile([P, NCHUNK], F32, tag="accM")
            accP = pp.tile([P, NCHUNK], F32, tag="accP")
            accA = pp.tile([P, NCHUNK], F32, tag="accA")
            R16 = pp.tile([P, 3, 3, S], F16, tag="R16")
            V16 = pp.tile([P, 3, S], F16, tag="V16")

            def bconst(val, name):
                t = pp.tile([P, 1], F32, tag=name, name=name)
                nc.gpsimd.memset(t[:], val)
                return t
            b_pih = bconst(PI / 2.0, "b_pih")
            b_pih23 = bconst(PI / 2.0 + 2.0 * PI / 3.0, "b_pih23")
            dbg = pp.tile([P, 8, S], F32, tag="dbg", name="dbg") if DEBUG else None

            # ---------------- pass 1: stream, mpjpe/accel/G/SP/ST ----------
            with tc.tile_pool(name="load1", bufs=2) as loadp, \
                 tc.tile_pool(name="half1", bufs=3) as halfp, \
                 tc.tile_pool(name="work1", bufs=2) as workp:
                for ci in range(NCHUNK):
                    p16 = _load_convert(nc, loadp, halfp, pv, ci, "p", stage=pstage)
                    t16 = _load_convert(nc, loadp, halfp, tv, ci, "t", stage=tstage)

                    # mpjpe
                    d = workp.tile([P, 3, 14, NB], F16, tag="d", bufs=1)
                    nc.vector.tensor_tensor(d[:], p16[:], t16[:], op=AX.subtract)
                    _sum3sq_sqrt_acc(nc, workp, d, 14, accM[:, ci:ci + 1], "m")

                    # accel: p[j] - 2 p[j+1] + p[j+2]
                    ta = workp.tile([P, 3, 12, NB], F16, tag="ta", bufs=1)
                    nc.vector.tensor_scalar_mul(ta[:], p16[:, :, 1:13, :], -2.0)
                    ab = workp.tile([P, 3, 12, NB], F16, tag="ab", bufs=1)
                    nc.vector.tensor_tensor(ab[:], ta[:], p16[:, :, 0:12, :], op=AX.add)
                    nc.vector.tensor_tensor(ab[:], ab[:], p16[:, :, 2:14, :], op=AX.add)
                    _sum3sq_sqrt_acc(nc, workp, ab, 12, accA[:, ci:ci + 1], "a")

                    # SP / ST (sums over J)
                    cs = slice(ci * NB, (ci + 1) * NB)
                    _tree14(nc, workp, p16[:], SP[:, :, cs].unsqueeze(2), "sp", eng=nc.gpsimd)
                    _tree14(nc, workp, t16[:], ST[:, :, cs].unsqueeze(2), "st", eng=nc.gpsimd)

                    # G[i,k] = sum_j P[i,j] T[k,j]
                    prodG = workp.tile([P, 3, 3, 14, NB], F16, tag="prodG", bufs=1)
                    nc.vector.tensor_tensor(
                        prodG[:],
                        p16[:].unsqueeze(2).broadcast_to([P, 3, 3, 14, NB]),
                        t16[:].unsqueeze(1).broadcast_to([P, 3, 3, 14, NB]),
                        op=AX.mult)
                    _tree14(nc, workp, prodG[:], G[:, :, :, cs].unsqueeze(3), "g")

            # ---------------- FOAM + pass 3, software-pipelined -------------
            # FOAM runs in sample-halves (fp32 slab working set). The second
            # half shares a pool scope with pass 3 so its serial dependency
            # chains overlap with pass-3 streaming work on other samples.
            SF = 256

            def pass3_chunk(halfp, workp, ci):
                QS = [P, 3, 3, 14, NB]
                p16 = _load_staged(nc, halfp, pstage, ci, "p")
                t16 = _load_staged(nc, halfp, tstage, ci, "t", bufs=1)
                cs = slice(ci * NB, (ci + 1) * NB)
                prodQ = workp.tile(QS, F16, tag="prodQ", name="prodQ", bufs=1)
                nc.vector.tensor_tensor(
                    prodQ[:],
                    R16[:, :, :, cs].unsqueeze(3).broadcast_to(QS),
                    p16[:].unsqueeze(1).broadcast_to(QS), op=AX.mult)
                qv = workp.tile([P, 3, 14, NB], F16, tag="qv", name="qv", bufs=1)
                nc.vector.tensor_tensor(qv[:], prodQ[:, :, 0], prodQ[:, :, 1], op=AX.add)
                nc.vector.tensor_tensor(qv[:], qv[:], prodQ[:, :, 2], op=AX.add)
                dv = workp.tile([P, 3, 14, NB], F16, tag="dv", name="dv", bufs=1)
                nc.vector.tensor_tensor(dv[:], qv[:], t16[:], op=AX.subtract)
                nc.vector.tensor_tensor(
                    dv[:], dv[:],
                    V16[:, :, cs].unsqueeze(2).broadcast_to([P, 3, 14, NB]),
                    op=AX.add)
                _sum3sq_sqrt_acc(nc, workp, dv, 14, accP[:, ci:ci + 1], "pa")

            if PHASES >= 2:
              with tc.tile_pool(name="slab", bufs=5) as sp_, \
                 tc.tile_pool(name="chs", bufs=22) as chp:
                _foam(nc, sp_, chp, G, SP, ST, R16, V16,
                      b_pih, b_pih23, 0, SF, dbg=dbg)

            if PHASES >= 2:
              with tc.tile_pool(name="slab2", bufs=3) as sp_, \
                 tc.tile_pool(name="chs2", bufs=10) as chp, \
                 tc.tile_pool(name="half3", bufs=2) as halfp, \
                 tc.tile_pool(name="work3", bufs=2) as workp:
                _foam(nc, sp_, chp, G, SP, ST, R16, V16,
                      b_pih, b_pih23, SF, SF, dbg=dbg)
                if PHASES >= 3:
                    for ci in range(NCHUNK):
                        pass3_chunk(halfp, workp, ci)

            stage = pp.tile([P, 24], F32, tag="stage", name="stage")
            nc.gpsimd.tensor_copy(stage[:, 0:NCHUNK], accM[:])
            if PHASES >= 3:
                nc.gpsimd.tensor_copy(stage[:, NCHUNK:2 * NCHUNK], accP[:])
            nc.gpsimd.tensor_copy(stage[:, 2 * NCHUNK:3 * NCHUNK], accA[:])
            nc.sync.dma_start(out[:], stage[:])
            if DEBUG:
                nc.sync.dma_start(dbg_t[:].rearrange("p (c s) -> p c s", c=8), dbg[:])
                rstage = pp.tile([P, 9, S], F32, tag="rstage", name="rstage")
                nc.gpsimd.tensor_copy(rstage[:], R16[:].rearrange("p a b s -> p (a b) s"))
                nc.sync.dma_start(dbgr_t[:].rearrange("p (c s) -> p c s", c=9), rstage[:])

    nc.finalize()
    return nc


_NC = None


def kernel(pred: np.ndarray, target: np.ndarray) -> np.ndarray:
    global _NC
    if _NC is None:
        _NC = build_bass()

    pred = np.ascontiguousarray(pred, dtype=np.float32).reshape(B_FULL, CJ)
    target = np.ascontiguousarray(target, dtype=np.float32).reshape(B_FULL, CJ)

    in_maps = []
    for c in range(N_CORES):
        sl = slice(c * B_LOC, (c + 1) * B_LOC)
        in_maps.append({"pred": pred[sl], "target": target[sl]})

    res = run_bass_kernel_spmd(_NC, in_maps, core_ids=list(range(N_CORES)))
    mp = pa = ac = 0.0
    for r in res.results:
        o = r["out"].astype(np.float64)
        mp += o[:, 0:NCHUNK].sum()
        pa += o[:, NCHUNK:2 * NCHUNK].sum()
        ac += o[:, 2 * NCHUNK:3 * NCHUNK].sum()
    return np.array([mp / (B_FULL * 14), pa / (B_FULL * 14), ac / (B_FULL * 12)],
                    dtype=np.float32)



# revision 12
# speedup vs baseline: 1.1485x; 1.1485x over previous
"""PoseMetrics (mpjpe / pa_mpjpe / accel_error) Trainium2 Bass kernel.

Full inputs: pred/target [524288, 3, 14] fp32. Output: [3] fp32.

Strategy (pure data parallel, 8 cores x 65536 samples):
  - Layout: 128 partitions x 512 samples/partition, samples innermost so the
    bulk fp16 elementwise work hits the DVE 2x mode. Inputs are converted
    once to persistent fp16 SBUF tiles (with a global 1/sqrt(8) prescale) and
    never re-streamed.
  - The tensor engine (PE) acts as a free accumulator: identity-weight
    matmuls into PSUM replace the j-sum trees (cross-covariance G, joint sums
    SP/ST) and the 3-way coordinate sums for the per-joint norms.
  - Kabsch/SVD is replaced by a closed form: K = H^T H, largest eigenvalue
    via cubic Newton (Cardano-bound start, 2 iters), remaining eigenvalues by
    quadratic deflation, lambda = s1+s2+sign(det H)*s3, then Markley's FOAM
    formula for R. Slab math fp16, eigen chain fp32.
  - Each core returns [128, 48] partial sums; host reduces in float64.
"""

import numpy as np

import concourse.bass as bass
import concourse.bacc as bacc
import concourse.mybir as mybir
import concourse.tile as tile
from concourse.bass_utils import run_bass_kernel_spmd
from concourse.masks import make_identity

F32 = mybir.dt.float32
F16 = mybir.dt.float16
AX = mybir.AluOpType
AF = mybir.ActivationFunctionType

N_CORES = 8
B_FULL = 524288
B_LOC = B_FULL // N_CORES          # 65536
P = 128                            # partitions
S = B_LOC // P                     # 512 samples per partition
NB = 64                            # samples per chunk (per partition)
NCHUNK = S // NB                   # 8
CJ = 42                            # 3*14
SF = 256                           # FOAM half size
SCALE = float(1.0 / np.sqrt(8.0))  # global input prescale (folded out on host)
SQ14I = float(1.0 / np.sqrt(14.0))
NACC = 2 * NCHUNK                  # accum slots per metric (2 PSUM subs/chunk)


def _pass1_chunk(nc, loadp, workp, pv, tv, p16, t16, Gp2, Gp1, SPp, STp,
                 n2M, n2A, accM, accA, G16, SPh, STh, I16, ci):
    cs = slice(ci * NB, (ci + 1) * NB)
    x32p = loadp.tile([P, NB, CJ], F32, tag="p32", name="x32p")
    nc.sync.dma_start(x32p[:], pv[:, cs, :])
    x32t = loadp.tile([P, NB, CJ], F32, tag="t32", name="x32t")
    nc.sync.dma_start(x32t[:], tv[:, cs, :])

    # fp32 -> fp16 J-major convert with the global prescale folded in
    nc.scalar.activation(p16[:, :, :, cs],
                         x32p[:].rearrange("p s (c j) -> p c j s", c=3, j=14),
                         AF.Copy, scale=SCALE)
    nc.scalar.activation(t16[:, :, :, cs],
                         x32t[:].rearrange("p s (c j) -> p c j s", c=3, j=14),
                         AF.Copy, scale=SCALE)

    pcs = p16[:, :, :, cs]
    tcs = t16[:, :, :, cs]

    # ---- mpjpe: d, d^2, PE c-sum, sqrt-accum --------------------------------
    d = workp.tile([P, 3, 14, NB], F16, tag="d", name="d")
    nc.vector.tensor_tensor(d[:], pcs, tcs, op=AX.subtract)
    d2 = workp.tile([P, 3, 14, NB], F16, tag="d2", name="d2")
    nc.scalar.square(d2[:], d[:])
    for sub in range(2):
        ss = slice(sub * 32, sub * 32 + 32)
        for c in range(3):
            nc.tensor.matmul(n2M[sub][:], I16[:], d2[:, c, :, ss],
                             start=(c == 0), stop=(c == 2))
        scrM = workp.tile([P, 14, 32], F16, tag="scrM", name="scrM")
        nc.scalar.activation(scrM[:], n2M[sub][:], AF.Sqrt,
                             accum_out=accM[:, 2 * ci + sub:2 * ci + sub + 1])

    # ---- accel: second difference over j, squares, PE c-sum ----------------
    ta = workp.tile([P, 3, 12, NB], F16, tag="ta", name="ta")
    nc.vector.tensor_scalar_mul(ta[:], pcs[:, :, 1:13, :], -2.0)
    nc.vector.tensor_tensor(ta[:], ta[:], pcs[:, :, 0:12, :], op=AX.add)
    nc.vector.tensor_tensor(ta[:], ta[:], pcs[:, :, 2:14, :], op=AX.add)
    a2 = workp.tile([P, 3, 12, NB], F16, tag="a2", name="a2")
    nc.scalar.square(a2[:], ta[:])
    for sub in range(2):
        ss = slice(sub * 32, sub * 32 + 32)
        for c in range(3):
            nc.tensor.matmul(n2A[sub][:], I16[:], a2[:, c, :, ss],
                             start=(c == 0), stop=(c == 2))
        scrA = workp.tile([P, 12, 32], F16, tag="scrA", name="scrA")
        nc.scalar.activation(scrA[:], n2A[sub][:], AF.Sqrt,
                             accum_out=accA[:, 2 * ci + sub:2 * ci + sub + 1])

    # ---- G / SP / ST via PE -------------------------------------------------
    # prod[k, i, j, s] = p_i t_k; one TT per k keeps APs within 3 free dims.
    CS = [P, 3, 14, NB]
    prod = workp.tile([P, 3, 3, 14, NB], F16, tag="prod", name="prod")
    for k in range(3):
        nc.vector.tensor_tensor(
            prod[:, k], pcs,
            tcs[:, k].unsqueeze(1).broadcast_to(CS), op=AX.mult)
    # G16[k, i] = sum_j prod[k, i, j]; split k to fit PSUM banks
    for (gp, ksl, nk) in ((Gp2, slice(0, 2), 2), (Gp1, slice(2, 3), 1)):
        for j in range(14):
            nc.tensor.matmul(gp[:], I16[:], prod[:, ksl, :, j, :],
                             start=(j == 0), stop=(j == 13))
    for j in range(14):
        nc.tensor.matmul(SPp[:], I16[:], p16[:, :, j, cs],
                         start=(j == 0), stop=(j == 13))
    for j in range(14):
        nc.tensor.matmul(STp[:], I16[:], t16[:, :, j, cs],
                         start=(j == 0), stop=(j == 13))

    # drains: G + SP/ST on ACT (GPSIMD cannot read PSUM)
    nc.scalar.copy(G16[:, 0:2, :, cs], Gp2[:])
    nc.scalar.copy(G16[:, 2:3, :, cs], Gp1[:])
    nc.scalar.activation(SPh[:, :, cs], SPp[:], AF.Copy, scale=SQ14I)
    nc.scalar.activation(STh[:, :, cs], STp[:], AF.Copy, scale=SQ14I)


def _foam_half(nc, sp_, chp, G16, SPh, STh, R16, V16, hf):
    """FOAM rotation for sample half hf (SF samples per partition).

    H is in s^2 = 1/8 scale (inherited from the input prescale); the FOAM
    formula is scale-invariant so no rescaling is needed anywhere.
    SPh/STh are joint sums scaled by 1/sqrt(14).
    """
    fs = slice(hf * SF, hf * SF + SF)
    S3 = [P, 3, 3, SF]
    # G16 is stored (k, i); present it as (i, k) via a stride view
    Gv = G16[:, :, :, fs].transpose([0, 2, 1, 3])
    SPv = SPh[:, :, fs]
    STv = STh[:, :, fs]

    def slab(name):
        # rotating scratch slab; at most `bufs` of these live at once
        return sp_.tile(S3, F16, tag="ktmp", name=name)

    def ch(name, dt=F32):
        return chp.tile([P, SF], dt, tag="ch32" if dt == F32 else "ch16",
                        name=name)

    def named(tag, dt=F32):
        return chp.tile([P, SF], dt, tag=tag, name=tag, bufs=1)

    # H = G - SP ST^T / 14  (SPh*STh = SP*ST/14 already)
    outer = slab("outer")
    nc.vector.tensor_tensor(
        outer[:], SPv.unsqueeze(2).broadcast_to(S3),
        STv.unsqueeze(1).broadcast_to(S3), op=AX.mult)
    H16 = sp_.tile(S3, F16, tag="H16", bufs=1, name="H16")
    nc.vector.tensor_tensor(H16[:], Gv, outer[:], op=AX.subtract)

    # detH on Pool (fp32 out), from fp16 H
    detH = named("detH")
    c1 = ch("det_c1"); c2 = ch("det_c2"); acc = ch("det_acc")
    nc.gpsimd.tensor_tensor(c1[:], H16[:, 1, 1], H16[:, 2, 2], op=AX.mult)
    nc.gpsimd.tensor_tensor(c2[:], H16[:, 1, 2], H16[:, 2, 1], op=AX.mult)
    nc.gpsimd.tensor_tensor(c1[:], c1[:], c2[:], op=AX.subtract)
    nc.gpsimd.tensor_tensor(acc[:], H16[:, 0, 0], c1[:], op=AX.mult)
    nc.gpsimd.tensor_tensor(c1[:], H16[:, 1, 0], H16[:, 2, 2], op=AX.mult)
    nc.gpsimd.tensor_tensor(c2[:], H16[:, 1, 2], H16[:, 2, 0], op=AX.mult)
    nc.gpsimd.tensor_tensor(c1[:], c1[:], c2[:], op=AX.subtract)
    nc.gpsimd.tensor_tensor(c1[:], H16[:, 0, 1], c1[:], op=AX.mult)
    nc.gpsimd.tensor_tensor(acc[:], acc[:], c1[:], op=AX.subtract)
    nc.gpsimd.tensor_tensor(c1[:], H16[:, 1, 0], H16[:, 2, 1], op=AX.mult)
    nc.gpsimd.tensor_tensor(c2[:], H16[:, 1, 1], H16[:, 2, 0], op=AX.mult)
    nc.gpsimd.tensor_tensor(c1[:], c1[:], c2[:], op=AX.subtract)
    nc.gpsimd.tensor_tensor(c1[:], H16[:, 0, 2], c1[:], op=AX.mult)
    nc.gpsimd.tensor_tensor(detH[:], acc[:], c1[:], op=AX.add)

    # K = H^T H (fp16 slabs, accumulate into K16 with one rotating temp)
    K16 = sp_.tile(S3, F16, tag="K16", bufs=1, name="K16")
    nc.vector.tensor_tensor(K16[:], H16[:, 0].unsqueeze(2).broadcast_to(S3),
                            H16[:, 0].unsqueeze(1).broadcast_to(S3), op=AX.mult)
    for c in (1, 2):
        tc_ = slab(f"t{c}")
        nc.vector.tensor_tensor(tc_[:], H16[:, c].unsqueeze(2).broadcast_to(S3),
                                H16[:, c].unsqueeze(1).broadcast_to(S3),
                                op=AX.mult)
        nc.vector.tensor_tensor(K16[:], K16[:], tc_[:], op=AX.add)

    # invariants: m2 = tr K (fp32), I3 = detH^2, I2 via Pool
    m2 = named("m2")
    nc.vector.tensor_tensor(m2[:], K16[:, 0, 0], K16[:, 1, 1], op=AX.add)
    nc.vector.tensor_tensor(m2[:], m2[:], K16[:, 2, 2], op=AX.add)
    I3 = named("I3")
    nc.vector.tensor_tensor(I3[:], detH[:], detH[:], op=AX.mult)

    o01 = ch("o01"); o02 = ch("o02"); o12 = ch("o12")
    nc.scalar.square(o01[:], K16[:, 0, 1])
    nc.scalar.square(o02[:], K16[:, 0, 2])
    nc.scalar.square(o12[:], K16[:, 1, 2])
    I2 = named("I2"); mm = ch("mm")
    nc.gpsimd.tensor_tensor(I2[:], K16[:, 0, 0], K16[:, 1, 1], op=AX.mult)
    nc.gpsimd.tensor_tensor(I2[:], I2[:], o01[:], op=AX.subtract)
    nc.gpsimd.tensor_tensor(mm[:], K16[:, 0, 0], K16[:, 2, 2], op=AX.mult)
    nc.gpsimd.tensor_tensor(mm[:], mm[:], o02[:], op=AX.subtract)
    nc.gpsimd.tensor_tensor(I2[:], I2[:], mm[:], op=AX.add)
    nc.gpsimd.tensor_tensor(mm[:], K16[:, 1, 1], K16[:, 2, 2], op=AX.mult)
    nc.gpsimd.tensor_tensor(mm[:], mm[:], o12[:], op=AX.subtract)
    nc.gpsimd.tensor_tensor(I2[:], I2[:], mm[:], op=AX.add)

    # Cardano upper bound start: x0 = m2/3 + 2*sqrt((dsum + 2*osum)/6)
    q = named("q")
    nc.vector.tensor_scalar_mul(q[:], m2[:], 1.0 / 3.0)
    osum = ch("osum")
    nc.vector.tensor_tensor(osum[:], o01[:], o02[:], op=AX.add)
    nc.vector.tensor_tensor(osum[:], osum[:], o12[:], op=AX.add)
    dsum = ch("dsum"); kd = ch("kd"); kd2 = ch("kd2")
    nc.vector.tensor_tensor(kd[:], K16[:, 0, 0], q[:], op=AX.subtract)
    nc.vector.tensor_tensor(dsum[:], kd[:], kd[:], op=AX.mult)
    nc.vector.tensor_tensor(kd[:], K16[:, 1, 1], q[:], op=AX.subtract)
    nc.vector.tensor_tensor(kd2[:], kd[:], kd[:], op=AX.mult)
    nc.vector.tensor_tensor(dsum[:], dsum[:], kd2[:], op=AX.add)
    nc.vector.tensor_tensor(kd[:], K16[:, 2, 2], q[:], op=AX.subtract)
    nc.vector.tensor_tensor(kd2[:], kd[:], kd[:], op=AX.mult)
    nc.vector.tensor_tensor(dsum[:], dsum[:], kd2[:], op=AX.add)
    p2 = ch("p2")
    nc.vector.scalar_tensor_tensor(p2[:], osum[:], 2.0, dsum[:],
                                   op0=AX.mult, op1=AX.add)
    pC = ch("pC")
    nc.scalar.activation(pC[:], p2[:], AF.Sqrt, scale=1.0 / 6.0)
    X = named("X")
    nc.vector.scalar_tensor_tensor(X[:], pC[:], 2.0, q[:],
                                   op0=AX.mult, op1=AX.add)

    # Newton on f(x) = ((x - m2) x + I2) x - I3, 2 iters from above
    m2_2 = named("m2_2")
    nc.vector.tensor_scalar_mul(m2_2[:], m2[:], 2.0)
    na = ch("na"); nb = ch("nb")
    for _ in range(2):
        nc.vector.tensor_tensor(na[:], X[:], m2[:], op=AX.subtract)
        nc.vector.tensor_tensor(na[:], na[:], X[:], op=AX.mult)
        nc.vector.tensor_tensor(na[:], na[:], I2[:], op=AX.add)
        nc.vector.tensor_tensor(na[:], na[:], X[:], op=AX.mult)
        nc.vector.tensor_tensor(na[:], na[:], I3[:], op=AX.subtract)   # f
        nc.vector.tensor_scalar_mul(nb[:], X[:], 3.0)
        nc.vector.tensor_tensor(nb[:], nb[:], m2_2[:], op=AX.subtract)
        nc.vector.tensor_tensor(nb[:], nb[:], X[:], op=AX.mult)
        nc.vector.tensor_tensor(nb[:], nb[:], I2[:], op=AX.add)        # f'
        nc.vector.reciprocal(nb[:], nb[:])
        nc.vector.tensor_tensor(na[:], na[:], nb[:], op=AX.mult)
        nc.vector.tensor_tensor(X[:], X[:], na[:], op=AX.subtract)

    # deflate: mu2/mu3 from x^2 - (m2-mu1)x + I3/mu1
    mus = chp.tile([P, 3, SF], F32, tag="mus", name="mus", bufs=1)
    mu1 = mus[:, 0]; mu2 = mus[:, 1]; mu3 = mus[:, 2]
    nc.vector.tensor_scalar_max(mu1, X[:], 1e-25)
    b = ch("b"); cc = ch("cc"); rmu = ch("rmu")
    nc.vector.tensor_tensor(b[:], m2[:], mu1, op=AX.subtract)
    nc.vector.reciprocal(rmu[:], mu1)
    nc.vector.tensor_tensor(cc[:], I3[:], rmu[:], op=AX.mult)
    b2 = ch("b2")
    nc.vector.tensor_tensor(b2[:], b[:], b[:], op=AX.mult)
    disc2 = ch("disc2")
    nc.vector.scalar_tensor_tensor(disc2[:], cc[:], -4.0, b2[:],
                                   op0=AX.mult, op1=AX.add)
    nc.vector.tensor_scalar_max(disc2[:], disc2[:], 0.0)
    disc = ch("disc")
    nc.scalar.sqrt(disc[:], disc2[:])
    bh = ch("bh")
    nc.vector.tensor_scalar_mul(bh[:], b[:], 0.5)
    nc.vector.scalar_tensor_tensor(mu2, disc[:], 0.5, bh[:],
                                   op0=AX.mult, op1=AX.add)
    nc.vector.tensor_scalar_max(mu2, mu2, 0.0)
    nc.vector.tensor_tensor(mu3, b[:], mu2, op=AX.subtract)
    nc.vector.tensor_scalar_max(mu3, mu3, 0.0)

    rt = chp.tile([P, 3, SF], F32, tag="rt", name="rt", bufs=1)
    nc.scalar.sqrt(rt[:], mus[:])
    sgn = ch("sgn")
    nc.scalar.sign(sgn[:], detH[:])
    lam = named("lam")
    nc.vector.tensor_tensor(lam[:], rt[:, 0], rt[:, 1], op=AX.add)
    s3s = ch("s3s")
    nc.vector.tensor_tensor(s3s[:], sgn[:], rt[:, 2], op=AX.mult)
    nc.vector.tensor_tensor(lam[:], lam[:], s3s[:], op=AX.add)

    # alpha2 = lam^2 + m2 ; zeta2 = (lam^2 - m2) lam - 2 detH (floored)
    lam2 = ch("lam2"); alpha2 = named("alpha2")
    nc.vector.tensor_tensor(lam2[:], lam[:], lam[:], op=AX.mult)
    nc.vector.tensor_tensor(alpha2[:], lam2[:], m2[:], op=AX.add)
    zt = ch("zt")
    nc.vector.tensor_tensor(zt[:], lam2[:], m2[:], op=AX.subtract)
    nc.vector.tensor_tensor(zt[:], zt[:], lam[:], op=AX.mult)
    zeta2 = ch("zeta2")
    nc.vector.scalar_tensor_tensor(zeta2[:], detH[:], -2.0, zt[:],
                                   op0=AX.mult, op1=AX.add)
    m2s = ch("m2s")
    nc.scalar.sqrt(m2s[:], m2[:])
    zfl = ch("zfl")
    nc.vector.scalar_tensor_tensor(zfl[:], m2s[:], 1e-4, m2[:],
                                   op0=AX.mult, op1=AX.mult)
    nc.vector.tensor_tensor(zeta2[:], zeta2[:], zfl[:], op=AX.max)
    rz = ch("rz")
    nc.vector.reciprocal(rz[:], zeta2[:])

    # fp16 stage for the slab assembly
    a16 = named("a16", F16)
    nc.vector.tensor_copy(a16[:], alpha2[:])
    l16 = named("l16", F16)
    nc.vector.tensor_scalar_mul(l16[:], lam[:], 2.0)
    rz16 = named("rz16", F16)
    nc.vector.tensor_copy(rz16[:], rz[:])

    # adjugate of H on Pool (fp16 out)
    adjH = sp_.tile(S3, F16, tag="adjH", bufs=1, name="adjH")
    idx = [
        (0, 0, (1, 1), (2, 2), (1, 2), (2, 1)),
        (0, 1, (0, 2), (2, 1), (0, 1), (2, 2)),
        (0, 2, (0, 1), (1, 2), (0, 2), (1, 1)),
        (1, 0, (1, 2), (2, 0), (1, 0), (2, 2)),
        (1, 1, (0, 0), (2, 2), (0, 2), (2, 0)),
        (1, 2, (0, 2), (1, 0), (0, 0), (1, 2)),
        (2, 0, (1, 0), (2, 1), (1, 1), (2, 0)),
        (2, 1, (0, 1), (2, 0), (0, 0), (2, 1)),
        (2, 2, (0, 0), (1, 1), (0, 1), (1, 0)),
    ]
    aw1 = ch("aw1"); aw2 = ch("aw2")
    for (i, j, (a1, a2), (b1, b2), (c1_, c2_), (d1, d2)) in idx:
        nc.gpsimd.tensor_tensor(aw1[:], H16[:, a1, a2], H16[:, b1, b2], op=AX.mult)
        nc.gpsimd.tensor_tensor(aw2[:], H16[:, c1_, c2_], H16[:, d1, d2], op=AX.mult)
        nc.gpsimd.tensor_tensor(adjH[:, i, j], aw1[:], aw2[:], op=AX.subtract)

    # Mt = K H^T (fp16 slabs), accumulated in place
    Mt = sp_.tile(S3, F16, tag="Mt", bufs=1, name="Mt")
    nc.vector.tensor_tensor(Mt[:], K16[:, :, 0].unsqueeze(2).broadcast_to(S3),
                            H16[:, :, 0].unsqueeze(1).broadcast_to(S3), op=AX.mult)
    for c in (1, 2):
        uc = slab(f"u{c}")
        nc.vector.tensor_tensor(uc[:], K16[:, :, c].unsqueeze(2).broadcast_to(S3),
                                H16[:, :, c].unsqueeze(1).broadcast_to(S3),
                                op=AX.mult)
        nc.vector.tensor_tensor(Mt[:], Mt[:], uc[:], op=AX.add)

    # num = alpha2 H^T + 2 lam adjH - 2 Mt ;  R = num / zeta2, clamped
    Ht = H16[:].transpose([0, 2, 1, 3])
    tB = slab("tB")
    nc.vector.tensor_tensor(
        tB[:], a16[:].unsqueeze(1).unsqueeze(2).broadcast_to(S3), Ht, op=AX.mult)
    vB = slab("vB")
    nc.vector.tensor_tensor(
        vB[:], l16[:].unsqueeze(1).unsqueeze(2).broadcast_to(S3), adjH[:],
        op=AX.mult)
    nc.vector.tensor_tensor(tB[:], tB[:], vB[:], op=AX.add)
    nc.vector.tensor_scalar_mul(Mt[:], Mt[:], -2.0)
    nc.vector.tensor_tensor(Mt[:], Mt[:], tB[:], op=AX.add)
    R16v = R16[:, :, :, fs]
    nc.vector.tensor_tensor(
        R16v, Mt[:], rz16[:].unsqueeze(1).unsqueeze(2).broadcast_to(S3),
        op=AX.mult)
    nc.vector.tensor_scalar(R16v, R16v, 4.0, -4.0, op0=AX.min, op1=AX.max)

    # V = (STh - R SPh) / sqrt(14)  (== t_mean - R p_mean)
    pv_ = slab("pv_")
    nc.vector.tensor_tensor(pv_[:], R16v, SPv.unsqueeze(1).broadcast_to(S3),
                            op=AX.mult)
    RS = chp.tile([P, 3, SF], F16, tag="RS", name="RS", bufs=1)
    nc.vector.tensor_tensor(RS[:], pv_[:, :, 0], pv_[:, :, 1], op=AX.add)
    nc.vector.tensor_tensor(RS[:], RS[:], pv_[:, :, 2], op=AX.add)
    Vt = chp.tile([P, 3, SF], F16, tag="Vt", name="Vt", bufs=1)
    nc.vector.tensor_tensor(Vt[:], STv, RS[:], op=AX.subtract)
    nc.vector.tensor_scalar_mul(V16[:, :, fs], Vt[:], SQ14I)


def _pass3_chunk(nc, workp, p16, t16, R16, V16, n2P, accP, I16, ci):
    cs = slice(ci * NB, (ci + 1) * NB)
    CS = [P, 3, 14, NB]
    # prq[k][i, j, s] = R_ik p_kj ; accumulate qv = sum_k in fp16
    qv = workp.tile(CS, F16, tag="qv", name="qv")
    nc.vector.tensor_tensor(
        qv[:], R16[:, :, 0, cs].unsqueeze(2).broadcast_to(CS),
        p16[:, 0, :, cs].unsqueeze(1).broadcast_to(CS), op=AX.mult)
    for k in (1, 2):
        prq = workp.tile(CS, F16, tag="prq", name="prq")
        nc.vector.tensor_tensor(
            prq[:], R16[:, :, k, cs].unsqueeze(2).broadcast_to(CS),
            p16[:, k, :, cs].unsqueeze(1).broadcast_to(CS), op=AX.mult)
        nc.vector.tensor_tensor(qv[:], qv[:], prq[:], op=AX.add)
    dv = workp.tile(CS, F16, tag="dv", name="dv")
    nc.vector.tensor_tensor(dv[:], qv[:], t16[:, :, :, cs], op=AX.subtract)
    nc.vector.tensor_tensor(
        dv[:], dv[:],
        V16[:, :, cs].unsqueeze(2).broadcast_to(CS), op=AX.add)
    dv2 = workp.tile([P, 3, 14, NB], F16, tag="dv2", name="dv2")
    nc.scalar.square(dv2[:], dv[:])
    for sub in range(2):
        ss = slice(sub * 32, sub * 32 + 32)
        for c in range(3):
            nc.tensor.matmul(n2P[sub][:], I16[:], dv2[:, c, :, ss],
                             start=(c == 0), stop=(c == 2))
        scrP = workp.tile([P, 14, 32], F16, tag="scrP", name="scrP")
        nc.scalar.activation(scrP[:], n2P[sub][:], AF.Sqrt,
                             accum_out=accP[:, 2 * ci + sub:2 * ci + sub + 1])


def build_bass():
    nc = bacc.Bacc("TRN2")
    pred = nc.dram_tensor("pred", [B_LOC, CJ], F32, kind="ExternalInput")
    targ = nc.dram_tensor("target", [B_LOC, CJ], F32, kind="ExternalInput")
    out = nc.dram_tensor("out", [P, 3 * NACC], F32, kind="ExternalOutput")

    pv = pred[:].rearrange("(p n) d -> p n d", p=P)   # [128, 512, 42]
    tv = targ[:].rearrange("(p n) d -> p n d", p=P)

    with tile.TileContext(nc) as tc:
        with tc.tile_pool(name="persist", bufs=1) as pp:
            p16 = pp.tile([P, 3, 14, S], F16, tag="p16")
            t16 = pp.tile([P, 3, 14, S], F16, tag="t16")
            G16 = pp.tile([P, 3, 3, S], F16, tag="G16")
            SPh = pp.tile([P, 3, S], F16, tag="SPh")
            STh = pp.tile([P, 3, S], F16, tag="STh")
            R16 = pp.tile([P, 3, 3, S], F16, tag="R16")
            V16 = pp.tile([P, 3, S], F16, tag="V16")
            accM = pp.tile([P, NACC], F32, tag="accM")
            accA = pp.tile([P, NACC], F32, tag="accA")
            accP = pp.tile([P, NACC], F32, tag="accP")
            I16 = pp.tile([P, P], F16, tag="I16")
            make_identity(nc, I16[:])

            # ---------------- pass 1 ----------------------------------------
            with tc.tile_pool(name="load1", bufs=2) as loadp, \
                 tc.tile_pool(name="work1", bufs=1) as workp, \
                 tc.tile_pool(name="ps1", bufs=1, space="PSUM") as psp:
                Gp2 = psp.tile([P, 2, 3, NB], F32, tag="Gp2")
                Gp1 = psp.tile([P, 1, 3, NB], F32, tag="Gp1")
                SPp = psp.tile([P, 3, NB], F32, tag="SPp")
                STp = psp.tile([P, 3, NB], F32, tag="STp")
                n2M = [psp.tile([P, 14, 32], F32, tag=f"n2M{s}", name=f"n2M{s}")
                       for s in range(2)]
                n2A = [psp.tile([P, 12, 32], F32, tag=f"n2A{s}", name=f"n2A{s}")
                       for s in range(2)]
                for ci in range(NCHUNK):
                    _pass1_chunk(nc, loadp, workp, pv, tv, p16, t16,
                                 Gp2, Gp1, SPp, STp, n2M, n2A,
                                 accM, accA, G16, SPh, STh, I16, ci)

            # ---------------- FOAM halves -----------------------------------
            with tc.tile_pool(name="slab_a", bufs=2) as sp_a, \
                 tc.tile_pool(name="ch_a", bufs=14) as chp_a:
                _foam_half(nc, sp_a, chp_a, G16, SPh, STh, R16, V16, 0)
                _foam_half(nc, sp_a, chp_a, G16, SPh, STh, R16, V16, 1)

            # ---------------- pass 3 ----------------------------------------
            with tc.tile_pool(name="work3", bufs=1) as workp3, \
                 tc.tile_pool(name="ps3", bufs=1, space="PSUM") as psp3:
                n2P = [psp3.tile([P, 14, 32], F32, tag=f"n2P{s}", name=f"n2P{s}")
                       for s in range(2)]
                for ci in range(NCHUNK):
                    _pass3_chunk(nc, workp3, p16, t16, R16, V16, n2P,
                                 accP, I16, ci)

            stage = pp.tile([P, 3 * NACC], F32, tag="stage", name="stage")
            nc.gpsimd.tensor_copy(stage[:, 0:NACC], accM[:])
            nc.gpsimd.tensor_copy(stage[:, NACC:2 * NACC], accP[:])
            nc.gpsimd.tensor_copy(stage[:, 2 * NACC:3 * NACC], accA[:])
            nc.sync.dma_start(out[:], stage[:])

    nc.finalize()
    return nc


_NC = None


def kernel(pred: np.ndarray, target: np.ndarray) -> np.ndarray:
    global _NC
    if _NC is None:
        _NC = build_bass()

    pred = np.ascontiguousarray(pred, dtype=np.float32).reshape(B_FULL, CJ)
    target = np.ascontiguousarray(target, dtype=np.float32).reshape(B_FULL, CJ)

    in_maps = []
    for c in range(N_CORES):
        sl = slice(c * B_LOC, (c + 1) * B_LOC)
        in_maps.append({"pred": pred[sl], "target": target[sl]})

    res = run_bass_kernel_spmd(_NC, in_maps, core_ids=list(range(N_CORES)))
    mp = pa = ac = 0.0
    for r in res.results:
        o = r["out"].astype(np.float64)
        mp += o[:, 0:NACC].sum()
        pa += o[:, NACC:2 * NACC].sum()
        ac += o[:, 2 * NACC:3 * NACC].sum()
    inv = 1.0 / SCALE
    return np.array([mp / (B_FULL * 14) * inv,
                     pa / (B_FULL * 14) * inv,
                     ac / (B_FULL * 12) * inv], dtype=np.float32)


# revision 18
# speedup vs baseline: 1.3519x; 1.1771x over previous
"""PoseMetrics (mpjpe / pa_mpjpe / accel_error) Trainium2 Bass kernel.

Full inputs: pred/target [524288, 3, 14] fp32. Output: [3] fp32.

Strategy (pure data parallel, 8 cores x 65536 samples):
  - Layout: 128 partitions x 512 samples/partition, samples innermost so the
    bulk fp16 elementwise work hits the DVE 2x mode. Inputs are converted
    once to persistent fp16 SBUF tiles (with a global 1/sqrt(8) prescale) and
    never re-streamed.
  - The tensor engine (PE) acts as a free accumulator: identity-weight
    matmuls into PSUM replace the j-sum trees (cross-covariance G, joint sums
    SP/ST) and the 3-way coordinate sums for the per-joint norms.
  - Kabsch/SVD is replaced by a closed form: K = H^T H, largest eigenvalue
    via cubic Newton (Cardano-bound start, 2 iters), remaining eigenvalues by
    quadratic deflation, lambda = s1+s2+sign(det H)*s3, then Markley's FOAM
    formula for R. Slab math fp16, eigen chain fp32.
  - Each core returns [128, 48] partial sums; host reduces in float64.
"""

import numpy as np

import concourse.bass as bass
import concourse.bacc as bacc
import concourse.mybir as mybir
import concourse.tile as tile
from concourse.bass_utils import run_bass_kernel_spmd
from concourse.masks import make_identity

F32 = mybir.dt.float32
F16 = mybir.dt.float16
AX = mybir.AluOpType
AF = mybir.ActivationFunctionType

N_CORES = 8
B_FULL = 524288
B_LOC = B_FULL // N_CORES          # 65536
P = 128                            # partitions
S = B_LOC // P                     # 512 samples per partition
NB = 64                            # samples per chunk (per partition)
NCHUNK = S // NB                   # 8
CJ = 42                            # 3*14
SF = 256                           # FOAM half size
SCALE = float(1.0 / np.sqrt(8.0))  # global input prescale (folded out on host)
SQ14I = float(1.0 / np.sqrt(14.0))
NACC = 2 * NCHUNK                  # accum slots per metric (2 PSUM subs/chunk)


def _pass1_chunk(nc, loadp, workp, pv, tv, p16, t16, Gp2, Gp1, SPp, STp,
                 n2M, n2A, accM, accA, G16, SPh, STh, I16, ci):
    cs = slice(ci * NB, (ci + 1) * NB)
    x32p = loadp.tile([P, NB, CJ], F32, tag="p32", name="x32p")
    nc.sync.dma_start(x32p[:], pv[:, cs, :])
    x32t = loadp.tile([P, NB, CJ], F32, tag="t32", name="x32t")
    nc.sync.dma_start(x32t[:], tv[:, cs, :])

    # fp32 -> fp16 J-major convert with the global prescale folded in.
    # On Pool: ACT is the pass-1 critical engine, Pool is idle here.
    nc.gpsimd.tensor_scalar_mul(
        p16[:, :, :, cs],
        x32p[:].rearrange("p s (c j) -> p c j s", c=3, j=14), SCALE)
    nc.gpsimd.tensor_scalar_mul(
        t16[:, :, :, cs],
        x32t[:].rearrange("p s (c j) -> p c j s", c=3, j=14), SCALE)

    pcs = p16[:, :, :, cs]
    tcs = t16[:, :, :, cs]

    # ---- mpjpe: d, d^2, PE c-sum, sqrt-accum --------------------------------
    d = workp.tile([P, 3, 14, NB], F16, tag="d", name="d")
    nc.vector.tensor_tensor(d[:], pcs, tcs, op=AX.subtract)
    d2 = workp.tile([P, 3, 14, NB], F16, tag="d2", name="d2")
    nc.scalar.square(d2[:], d[:])
    for sub in range(2):
        ss = slice(sub * 32, sub * 32 + 32)
        for c in range(3):
            nc.tensor.matmul(n2M[sub][:], I16[:], d2[:, c, :, ss],
                             start=(c == 0), stop=(c == 2))
        scrM = workp.tile([P, 14, 32], F16, tag="scrM", name="scrM")
        nc.scalar.activation(scrM[:], n2M[sub][:], AF.Sqrt,
                             accum_out=accM[:, 2 * ci + sub:2 * ci + sub + 1])

    # ---- accel: second difference over j, squares, PE c-sum ----------------
    ta = workp.tile([P, 3, 12, NB], F16, tag="ta", name="ta")
    nc.vector.tensor_scalar_mul(ta[:], pcs[:, :, 1:13, :], -2.0)
    nc.vector.tensor_tensor(ta[:], ta[:], pcs[:, :, 0:12, :], op=AX.add)
    nc.vector.tensor_tensor(ta[:], ta[:], pcs[:, :, 2:14, :], op=AX.add)
    a2 = workp.tile([P, 3, 12, NB], F16, tag="a2", name="a2")
    nc.scalar.square(a2[:], ta[:])
    for sub in range(2):
        ss = slice(sub * 32, sub * 32 + 32)
        for c in range(3):
            nc.tensor.matmul(n2A[sub][:], I16[:], a2[:, c, :, ss],
                             start=(c == 0), stop=(c == 2))
        scrA = workp.tile([P, 12, 32], F16, tag="scrA", name="scrA")
        nc.scalar.activation(scrA[:], n2A[sub][:], AF.Sqrt,
                             accum_out=accA[:, 2 * ci + sub:2 * ci + sub + 1])

    # ---- G / SP / ST via PE -------------------------------------------------
    # prod[k, i, j, s] = p_i t_k; one TT per k keeps APs within 3 free dims.
    CS = [P, 3, 14, NB]
    prod = workp.tile([P, 3, 3, 14, NB], F16, tag="prod", name="prod")
    for k in range(3):
        nc.vector.tensor_tensor(
            prod[:, k], pcs,
            tcs[:, k].unsqueeze(1).broadcast_to(CS), op=AX.mult)
    # G16[k, i] = sum_j prod[k, i, j]; split k to fit PSUM banks
    for (gp, ksl, nk) in ((Gp2, slice(0, 2), 2), (Gp1, slice(2, 3), 1)):
        for j in range(14):
            nc.tensor.matmul(gp[:], I16[:], prod[:, ksl, :, j, :],
                             start=(j == 0), stop=(j == 13))
    for j in range(14):
        nc.tensor.matmul(SPp[:], I16[:], p16[:, :, j, cs],
                         start=(j == 0), stop=(j == 13))
    for j in range(14):
        nc.tensor.matmul(STp[:], I16[:], t16[:, :, j, cs],
                         start=(j == 0), stop=(j == 13))

    # drains: G + SP/ST on ACT (GPSIMD cannot read PSUM)
    nc.scalar.copy(G16[:, 0:2, :, cs], Gp2[:])
    nc.scalar.copy(G16[:, 2:3, :, cs], Gp1[:])
    nc.scalar.activation(SPh[:, :, cs], SPp[:], AF.Copy, scale=SQ14I)
    nc.scalar.activation(STh[:, :, cs], STp[:], AF.Copy, scale=SQ14I)


def _foam_half_a(nc, sp_, chp, G16, SPh, STh, R16, V16, hf):
    """FOAM part A: H, K, invariants, eigen chain, adjugate, fp16 staging.

    H is in s^2 = 1/8 scale (inherited from the input prescale); the FOAM
    formula is scale-invariant so no rescaling is needed anywhere.
    SPh/STh are joint sums scaled by 1/sqrt(14).
    """
    fs = slice(hf * SF, hf * SF + SF)
    S3 = [P, 3, 3, SF]
    # G16 is stored (k, i); present it as (i, k) via a stride view
    Gv = G16[:, :, :, fs].transpose([0, 2, 1, 3])
    SPv = SPh[:, :, fs]
    STv = STh[:, :, fs]

    def slab(name):
        # rotating scratch slab; at most `bufs` of these live at once
        return sp_.tile(S3, F16, tag="ktmp", name=name)

    def ch(name, dt=F32):
        return chp.tile([P, SF], dt, tag="ch32" if dt == F32 else "ch16",
                        name=name)

    def named(tag, dt=F32):
        return chp.tile([P, SF], dt, tag=tag, name=tag, bufs=1)

    # H = G - SP ST^T / 14  (SPh*STh = SP*ST/14 already)
    outer = slab("outer")
    nc.vector.tensor_tensor(
        outer[:], SPv.unsqueeze(2).broadcast_to(S3),
        STv.unsqueeze(1).broadcast_to(S3), op=AX.mult)
    H16 = sp_.tile(S3, F16, tag="H16", bufs=1, name="H16")
    nc.vector.tensor_tensor(H16[:], Gv, outer[:], op=AX.subtract)

    # detH on Pool (fp32 out), from fp16 H
    detH = named("detH")
    c1 = ch("det_c1"); c2 = ch("det_c2"); acc = ch("det_acc")
    nc.gpsimd.tensor_tensor(c1[:], H16[:, 1, 1], H16[:, 2, 2], op=AX.mult)
    nc.gpsimd.tensor_tensor(c2[:], H16[:, 1, 2], H16[:, 2, 1], op=AX.mult)
    nc.gpsimd.tensor_tensor(c1[:], c1[:], c2[:], op=AX.subtract)
    nc.gpsimd.tensor_tensor(acc[:], H16[:, 0, 0], c1[:], op=AX.mult)
    nc.gpsimd.tensor_tensor(c1[:], H16[:, 1, 0], H16[:, 2, 2], op=AX.mult)
    nc.gpsimd.tensor_tensor(c2[:], H16[:, 1, 2], H16[:, 2, 0], op=AX.mult)
    nc.gpsimd.tensor_tensor(c1[:], c1[:], c2[:], op=AX.subtract)
    nc.gpsimd.tensor_tensor(c1[:], H16[:, 0, 1], c1[:], op=AX.mult)
    nc.gpsimd.tensor_tensor(acc[:], acc[:], c1[:], op=AX.subtract)
    nc.gpsimd.tensor_tensor(c1[:], H16[:, 1, 0], H16[:, 2, 1], op=AX.mult)
    nc.gpsimd.tensor_tensor(c2[:], H16[:, 1, 1], H16[:, 2, 0], op=AX.mult)
    nc.gpsimd.tensor_tensor(c1[:], c1[:], c2[:], op=AX.subtract)
    nc.gpsimd.tensor_tensor(c1[:], H16[:, 0, 2], c1[:], op=AX.mult)
    nc.gpsimd.tensor_tensor(detH[:], acc[:], c1[:], op=AX.add)

    # K = H^T H (fp16 slabs, accumulate into K16 with one rotating temp)
    K16 = sp_.tile(S3, F16, tag="K16", bufs=1, name="K16")
    nc.vector.tensor_tensor(K16[:], H16[:, 0].unsqueeze(2).broadcast_to(S3),
                            H16[:, 0].unsqueeze(1).broadcast_to(S3), op=AX.mult)
    for c in (1, 2):
        tc_ = slab(f"t{c}")
        nc.vector.tensor_tensor(tc_[:], H16[:, c].unsqueeze(2).broadcast_to(S3),
                                H16[:, c].unsqueeze(1).broadcast_to(S3),
                                op=AX.mult)
        nc.vector.tensor_tensor(K16[:], K16[:], tc_[:], op=AX.add)

    # invariants: m2 = tr K (fp32), I3 = detH^2, I2 via Pool
    m2 = named("m2")
    nc.vector.tensor_tensor(m2[:], K16[:, 0, 0], K16[:, 1, 1], op=AX.add)
    nc.vector.tensor_tensor(m2[:], m2[:], K16[:, 2, 2], op=AX.add)
    I3 = named("I3")
    nc.vector.tensor_tensor(I3[:], detH[:], detH[:], op=AX.mult)

    o01 = ch("o01"); o02 = ch("o02"); o12 = ch("o12")
    nc.scalar.square(o01[:], K16[:, 0, 1])
    nc.scalar.square(o02[:], K16[:, 0, 2])
    nc.scalar.square(o12[:], K16[:, 1, 2])
    I2 = named("I2"); mm = ch("mm")
    nc.gpsimd.tensor_tensor(I2[:], K16[:, 0, 0], K16[:, 1, 1], op=AX.mult)
    nc.gpsimd.tensor_tensor(I2[:], I2[:], o01[:], op=AX.subtract)
    nc.gpsimd.tensor_tensor(mm[:], K16[:, 0, 0], K16[:, 2, 2], op=AX.mult)
    nc.gpsimd.tensor_tensor(mm[:], mm[:], o02[:], op=AX.subtract)
    nc.gpsimd.tensor_tensor(I2[:], I2[:], mm[:], op=AX.add)
    nc.gpsimd.tensor_tensor(mm[:], K16[:, 1, 1], K16[:, 2, 2], op=AX.mult)
    nc.gpsimd.tensor_tensor(mm[:], mm[:], o12[:], op=AX.subtract)
    nc.gpsimd.tensor_tensor(I2[:], I2[:], mm[:], op=AX.add)

    # Cardano upper bound start: x0 = m2/3 + 2*sqrt((dsum + 2*osum)/6)
    q = named("q")
    nc.vector.tensor_scalar_mul(q[:], m2[:], 1.0 / 3.0)
    osum = ch("osum")
    nc.vector.tensor_tensor(osum[:], o01[:], o02[:], op=AX.add)
    nc.vector.tensor_tensor(osum[:], osum[:], o12[:], op=AX.add)
    dsum = ch("dsum"); kd = ch("kd"); kd2 = ch("kd2")
    nc.vector.tensor_tensor(kd[:], K16[:, 0, 0], q[:], op=AX.subtract)
    nc.vector.tensor_tensor(dsum[:], kd[:], kd[:], op=AX.mult)
    nc.vector.tensor_tensor(kd[:], K16[:, 1, 1], q[:], op=AX.subtract)
    nc.vector.tensor_tensor(kd2[:], kd[:], kd[:], op=AX.mult)
    nc.vector.tensor_tensor(dsum[:], dsum[:], kd2[:], op=AX.add)
    nc.vector.tensor_tensor(kd[:], K16[:, 2, 2], q[:], op=AX.subtract)
    nc.vector.tensor_tensor(kd2[:], kd[:], kd[:], op=AX.mult)
    nc.vector.tensor_tensor(dsum[:], dsum[:], kd2[:], op=AX.add)
    p2 = ch("p2")
    nc.vector.scalar_tensor_tensor(p2[:], osum[:], 2.0, dsum[:],
                                   op0=AX.mult, op1=AX.add)
    pC = ch("pC")
    nc.scalar.activation(pC[:], p2[:], AF.Sqrt, scale=1.0 / 6.0)
    X = named("X")
    nc.vector.scalar_tensor_tensor(X[:], pC[:], 2.0, q[:],
                                   op0=AX.mult, op1=AX.add)

    # Newton on f(x) = ((x - m2) x + I2) x - I3, 2 iters from above
    m2_2 = named("m2_2")
    nc.vector.tensor_scalar_mul(m2_2[:], m2[:], 2.0)
    na = ch("na"); nb = ch("nb")
    for _ in range(2):
        nc.vector.tensor_tensor(na[:], X[:], m2[:], op=AX.subtract)
        nc.vector.tensor_tensor(na[:], na[:], X[:], op=AX.mult)
        nc.vector.tensor_tensor(na[:], na[:], I2[:], op=AX.add)
        nc.vector.tensor_tensor(na[:], na[:], X[:], op=AX.mult)
        nc.vector.tensor_tensor(na[:], na[:], I3[:], op=AX.subtract)   # f
        nc.vector.tensor_scalar_mul(nb[:], X[:], 3.0)
        nc.vector.tensor_tensor(nb[:], nb[:], m2_2[:], op=AX.subtract)
        nc.vector.tensor_tensor(nb[:], nb[:], X[:], op=AX.mult)
        nc.vector.tensor_tensor(nb[:], nb[:], I2[:], op=AX.add)        # f'
        nc.vector.reciprocal(nb[:], nb[:])
        nc.vector.tensor_tensor(na[:], na[:], nb[:], op=AX.mult)
        nc.vector.tensor_tensor(X[:], X[:], na[:], op=AX.subtract)

    # deflate: mu2/mu3 from x^2 - (m2-mu1)x + I3/mu1
    mus = chp.tile([P, 3, SF], F32, tag="mus", name="mus", bufs=1)
    mu1 = mus[:, 0]; mu2 = mus[:, 1]; mu3 = mus[:, 2]
    nc.vector.tensor_scalar_max(mu1, X[:], 1e-25)
    b = ch("b"); cc = ch("cc"); rmu = ch("rmu")
    nc.vector.tensor_tensor(b[:], m2[:], mu1, op=AX.subtract)
    nc.vector.reciprocal(rmu[:], mu1)
    nc.vector.tensor_tensor(cc[:], I3[:], rmu[:], op=AX.mult)
    b2 = ch("b2")
    nc.vector.tensor_tensor(b2[:], b[:], b[:], op=AX.mult)
    disc2 = ch("disc2")
    nc.vector.scalar_tensor_tensor(disc2[:], cc[:], -4.0, b2[:],
                                   op0=AX.mult, op1=AX.add)
    nc.vector.tensor_scalar_max(disc2[:], disc2[:], 0.0)
    disc = ch("disc")
    nc.scalar.sqrt(disc[:], disc2[:])
    bh = ch("bh")
    nc.vector.tensor_scalar_mul(bh[:], b[:], 0.5)
    nc.vector.scalar_tensor_tensor(mu2, disc[:], 0.5, bh[:],
                                   op0=AX.mult, op1=AX.add)
    nc.vector.tensor_scalar_max(mu2, mu2, 0.0)
    nc.vector.tensor_tensor(mu3, b[:], mu2, op=AX.subtract)
    nc.vector.tensor_scalar_max(mu3, mu3, 0.0)

    rt = chp.tile([P, 3, SF], F32, tag="rt", name="rt", bufs=1)
    nc.scalar.sqrt(rt[:], mus[:])
    sgn = ch("sgn")
    nc.scalar.sign(sgn[:], detH[:])
    lam = named("lam")
    nc.vector.tensor_tensor(lam[:], rt[:, 0], rt[:, 1], op=AX.add)
    s3s = ch("s3s")
    nc.vector.tensor_tensor(s3s[:], sgn[:], rt[:, 2], op=AX.mult)
    nc.vector.tensor_tensor(lam[:], lam[:], s3s[:], op=AX.add)

    # alpha2 = lam^2 + m2 ; zeta2 = (lam^2 - m2) lam - 2 detH (floored)
    lam2 = ch("lam2"); alpha2 = named("alpha2")
    nc.vector.tensor_tensor(lam2[:], lam[:], lam[:], op=AX.mult)
    nc.vector.tensor_tensor(alpha2[:], lam2[:], m2[:], op=AX.add)
    zt = ch("zt")
    nc.vector.tensor_tensor(zt[:], lam2[:], m2[:], op=AX.subtract)
    nc.vector.tensor_tensor(zt[:], zt[:], lam[:], op=AX.mult)
    zeta2 = ch("zeta2")
    nc.vector.scalar_tensor_tensor(zeta2[:], detH[:], -2.0, zt[:],
                                   op0=AX.mult, op1=AX.add)
    m2s = ch("m2s")
    nc.scalar.sqrt(m2s[:], m2[:])
    zfl = ch("zfl")
    nc.vector.scalar_tensor_tensor(zfl[:], m2s[:], 1e-4, m2[:],
                                   op0=AX.mult, op1=AX.mult)
    nc.vector.tensor_tensor(zeta2[:], zeta2[:], zfl[:], op=AX.max)
    rz = ch("rz")
    nc.vector.reciprocal(rz[:], zeta2[:])

    # fp16 stage for the slab assembly
    a16 = named("a16", F16)
    nc.vector.tensor_copy(a16[:], alpha2[:])
    l16 = named("l16", F16)
    nc.vector.tensor_scalar_mul(l16[:], lam[:], 2.0)
    rz16 = named("rz16", F16)
    nc.vector.tensor_copy(rz16[:], rz[:])

    # adjugate of H on Pool (fp16 out)
    adjH = sp_.tile(S3, F16, tag="adjH", bufs=1, name="adjH")
    idx = [
        (0, 0, (1, 1), (2, 2), (1, 2), (2, 1)),
        (0, 1, (0, 2), (2, 1), (0, 1), (2, 2)),
        (0, 2, (0, 1), (1, 2), (0, 2), (1, 1)),
        (1, 0, (1, 2), (2, 0), (1, 0), (2, 2)),
        (1, 1, (0, 0), (2, 2), (0, 2), (2, 0)),
        (1, 2, (0, 2), (1, 0), (0, 0), (1, 2)),
        (2, 0, (1, 0), (2, 1), (1, 1), (2, 0)),
        (2, 1, (0, 1), (2, 0), (0, 0), (2, 1)),
        (2, 2, (0, 0), (1, 1), (0, 1), (1, 0)),
    ]
    aw1 = ch("aw1"); aw2 = ch("aw2")
    for (i, j, (a1, a2), (b1, b2), (c1_, c2_), (d1, d2)) in idx:
        nc.gpsimd.tensor_tensor(aw1[:], H16[:, a1, a2], H16[:, b1, b2], op=AX.mult)
        nc.gpsimd.tensor_tensor(aw2[:], H16[:, c1_, c2_], H16[:, d1, d2], op=AX.mult)
        nc.gpsimd.tensor_tensor(adjH[:, i, j], aw1[:], aw2[:], op=AX.subtract)

    return {"H16": H16, "K16": K16, "adjH": adjH,
            "a16": a16, "l16": l16, "rz16": rz16}


def _foam_half_b(nc, sp_, chp, st, G16, SPh, STh, R16, V16, t16, hf):
    """FOAM part B: slab assembly, R, V, and the V fold into t16."""
    fs = slice(hf * SF, hf * SF + SF)
    S3 = [P, 3, 3, SF]
    SPv = SPh[:, :, fs]
    STv = STh[:, :, fs]
    H16 = st["H16"]; K16 = st["K16"]; adjH = st["adjH"]
    a16 = st["a16"]; l16 = st["l16"]; rz16 = st["rz16"]

    def slab(name):
        return sp_.tile(S3, F16, tag="ktmp", name=name)

    # Mt = K H^T (fp16 slabs), accumulated in place
    Mt = sp_.tile(S3, F16, tag="Mt", bufs=1, name="Mt")
    nc.vector.tensor_tensor(Mt[:], K16[:, :, 0].unsqueeze(2).broadcast_to(S3),
                            H16[:, :, 0].unsqueeze(1).broadcast_to(S3), op=AX.mult)
    for c in (1, 2):
        uc = slab(f"u{c}")
        nc.vector.tensor_tensor(uc[:], K16[:, :, c].unsqueeze(2).broadcast_to(S3),
                                H16[:, :, c].unsqueeze(1).broadcast_to(S3),
                                op=AX.mult)
        nc.vector.tensor_tensor(Mt[:], Mt[:], uc[:], op=AX.add)

    # num = alpha2 H^T + 2 lam adjH - 2 Mt ;  R = num / zeta2, clamped
    Ht = H16[:].transpose([0, 2, 1, 3])
    tB = slab("tB")
    nc.vector.tensor_tensor(
        tB[:], a16[:].unsqueeze(1).unsqueeze(2).broadcast_to(S3), Ht, op=AX.mult)
    vB = slab("vB")
    nc.vector.tensor_tensor(
        vB[:], l16[:].unsqueeze(1).unsqueeze(2).broadcast_to(S3), adjH[:],
        op=AX.mult)
    nc.vector.tensor_tensor(tB[:], tB[:], vB[:], op=AX.add)
    nc.vector.tensor_scalar_mul(Mt[:], Mt[:], -2.0)
    nc.vector.tensor_tensor(Mt[:], Mt[:], tB[:], op=AX.add)
    R16v = R16[:, :, :, fs]
    nc.vector.tensor_tensor(
        R16v, Mt[:], rz16[:].unsqueeze(1).unsqueeze(2).broadcast_to(S3),
        op=AX.mult)
    nc.vector.tensor_scalar(R16v, R16v, 4.0, -4.0, op0=AX.min, op1=AX.max)

    # V = (STh - R SPh) / sqrt(14)  (== t_mean - R p_mean)
    pv_ = slab("pv_")
    nc.vector.tensor_tensor(pv_[:], R16v, SPv.unsqueeze(1).broadcast_to(S3),
                            op=AX.mult)
    RS = chp.tile([P, 3, SF], F16, tag="RS", name="RS", bufs=1)
    nc.vector.tensor_tensor(RS[:], pv_[:, :, 0], pv_[:, :, 1], op=AX.add)
    nc.vector.tensor_tensor(RS[:], RS[:], pv_[:, :, 2], op=AX.add)
    Vt = chp.tile([P, 3, SF], F16, tag="Vt", name="Vt", bufs=1)
    nc.vector.tensor_tensor(Vt[:], STv, RS[:], op=AX.subtract)
    nc.vector.tensor_scalar_mul(V16[:, :, fs], Vt[:], SQ14I)

    # fold V into t16 in place: pass3's residual becomes qv - t16
    TSH = [P, 3, 14, SF]
    nc.vector.tensor_tensor(
        t16[:, :, :, fs], t16[:, :, :, fs],
        V16[:, :, fs].unsqueeze(2).broadcast_to(TSH), op=AX.subtract)


def _pass3_chunk(nc, workp, p16, t16, R16, V16, n2P, accP, I16, ci):
    cs = slice(ci * NB, (ci + 1) * NB)
    CS = [P, 3, 14, NB]
    # prq[k][i, j, s] = R_ik p_kj ; accumulate qv = sum_k in fp16
    qv = workp.tile(CS, F16, tag="qv", name="qv")
    nc.vector.tensor_tensor(
        qv[:], R16[:, :, 0, cs].unsqueeze(2).broadcast_to(CS),
        p16[:, 0, :, cs].unsqueeze(1).broadcast_to(CS), op=AX.mult)
    for k in (1, 2):
        prq = workp.tile(CS, F16, tag="prq", name="prq")
        nc.vector.tensor_tensor(
            prq[:], R16[:, :, k, cs].unsqueeze(2).broadcast_to(CS),
            p16[:, k, :, cs].unsqueeze(1).broadcast_to(CS), op=AX.mult)
        nc.vector.tensor_tensor(qv[:], qv[:], prq[:], op=AX.add)
    # t16 already has V folded in (t16 <- t16 - V during FOAM)
    dv = workp.tile(CS, F16, tag="dv", name="dv")
    nc.vector.tensor_tensor(dv[:], qv[:], t16[:, :, :, cs], op=AX.subtract)
    dv2 = workp.tile([P, 3, 14, NB], F16, tag="dv2", name="dv2")
    nc.scalar.square(dv2[:], dv[:])
    for sub in range(2):
        ss = slice(sub * 32, sub * 32 + 32)
        for c in range(3):
            nc.tensor.matmul(n2P[sub][:], I16[:], dv2[:, c, :, ss],
                             start=(c == 0), stop=(c == 2))
        scrP = workp.tile([P, 14, 32], F16, tag="scrP", name="scrP")
        nc.scalar.activation(scrP[:], n2P[sub][:], AF.Sqrt,
                             accum_out=accP[:, 2 * ci + sub:2 * ci + sub + 1])


def build_bass():
    nc = bacc.Bacc("TRN2")
    pred = nc.dram_tensor("pred", [B_LOC, CJ], F32, kind="ExternalInput")
    targ = nc.dram_tensor("target", [B_LOC, CJ], F32, kind="ExternalInput")
    out = nc.dram_tensor("out", [P, 3 * NACC], F32, kind="ExternalOutput")

    pv = pred[:].rearrange("(p n) d -> p n d", p=P)   # [128, 512, 42]
    tv = targ[:].rearrange("(p n) d -> p n d", p=P)

    with tile.TileContext(nc) as tc:
        with tc.tile_pool(name="persist", bufs=1) as pp:
            p16 = pp.tile([P, 3, 14, S], F16, tag="p16")
            t16 = pp.tile([P, 3, 14, S], F16, tag="t16")
            G16 = pp.tile([P, 3, 3, S], F16, tag="G16")
            SPh = pp.tile([P, 3, S], F16, tag="SPh")
            STh = pp.tile([P, 3, S], F16, tag="STh")
            R16 = pp.tile([P, 3, 3, S], F16, tag="R16")
            V16 = pp.tile([P, 3, S], F16, tag="V16")
            accM = pp.tile([P, NACC], F32, tag="accM")
            accA = pp.tile([P, NACC], F32, tag="accA")
            accP = pp.tile([P, NACC], F32, tag="accP")
            I16 = pp.tile([P, P], F16, tag="I16")
            make_identity(nc, I16[:])

            # ---------------- pass 1 ----------------------------------------
            with tc.tile_pool(name="load1", bufs=2) as loadp, \
                 tc.tile_pool(name="work1", bufs=1) as workp, \
                 tc.tile_pool(name="ps1", bufs=1, space="PSUM") as psp:
                Gp2 = psp.tile([P, 2, 3, NB], F32, tag="Gp2")
                Gp1 = psp.tile([P, 1, 3, NB], F32, tag="Gp1")
                SPp = psp.tile([P, 3, NB], F32, tag="SPp")
                STp = psp.tile([P, 3, NB], F32, tag="STp")
                n2M = [psp.tile([P, 14, 32], F32, tag=f"n2M{s}", name=f"n2M{s}")
                       for s in range(2)]
                n2A = [psp.tile([P, 12, 32], F32, tag=f"n2A{s}", name=f"n2A{s}")
                       for s in range(2)]
                for ci in range(NCHUNK):
                    _pass1_chunk(nc, loadp, workp, pv, tv, p16, t16,
                                 Gp2, Gp1, SPp, STp, n2M, n2A,
                                 accM, accA, G16, SPh, STh, I16, ci)

            # ---------------- FOAM + pass 3, interleaved --------------------
            with tc.tile_pool(name="slab_a", bufs=2) as sp_a, \
                 tc.tile_pool(name="ch_a", bufs=14) as chp_a, \
                 tc.tile_pool(name="work3", bufs=1) as workp3, \
                 tc.tile_pool(name="ps3", bufs=1, space="PSUM") as psp3:
                n2P = [psp3.tile([P, 14, 32], F32, tag=f"n2P{s}", name=f"n2P{s}")
                       for s in range(2)]
                st0 = _foam_half_a(nc, sp_a, chp_a, G16, SPh, STh, R16, V16, 0)
                _foam_half_b(nc, sp_a, chp_a, st0, G16, SPh, STh, R16, V16,
                             t16, 0)
                st1 = _foam_half_a(nc, sp_a, chp_a, G16, SPh, STh, R16, V16, 1)
                _pass3_chunk(nc, workp3, p16, t16, R16, V16, n2P, accP, I16, 0)
                _foam_half_b(nc, sp_a, chp_a, st1, G16, SPh, STh, R16, V16,
                             t16, 1)
                for ci in range(1, NCHUNK):
                    _pass3_chunk(nc, workp3, p16, t16, R16, V16, n2P,
                                 accP, I16, ci)

            stage = pp.tile([P, 3 * NACC], F32, tag="stage", name="stage")
            nc.gpsimd.tensor_copy(stage[:, 0:NACC], accM[:])
            nc.gpsimd.tensor_copy(stage[:, NACC:2 * NACC], accP[:])
            nc.gpsimd.tensor_copy(stage[:, 2 * NACC:3 * NACC], accA[:])
            nc.sync.dma_start(out[:], stage[:])

    nc.finalize()
    return nc


_NC = None


def kernel(pred: np.ndarray, target: np.ndarray) -> np.ndarray:
    global _NC
    if _NC is None:
        _NC = build_bass()

    pred = np.ascontiguousarray(pred, dtype=np.float32).reshape(B_FULL, CJ)
    target = np.ascontiguousarray(target, dtype=np.float32).reshape(B_FULL, CJ)

    in_maps = []
    for c in range(N_CORES):
        sl = slice(c * B_LOC, (c + 1) * B_LOC)
        in_maps.append({"pred": pred[sl], "target": target[sl]})

    res = run_bass_kernel_spmd(_NC, in_maps, core_ids=list(range(N_CORES)))
    mp = pa = ac = 0.0
    for r in res.results:
        o = r["out"].astype(np.float64)
        mp += o[:, 0:NACC].sum()
        pa += o[:, NACC:2 * NACC].sum()
        ac += o[:, 2 * NACC:3 * NACC].sum()
    inv = 1.0 / SCALE
    return np.array([mp / (B_FULL * 14) * inv,
                     pa / (B_FULL * 14) * inv,
                     ac / (B_FULL * 12) * inv], dtype=np.float32)


# revision 24
# speedup vs baseline: 1.4783x; 1.0935x over previous
"""PoseMetrics (mpjpe / pa_mpjpe / accel_error) Trainium2 Bass kernel.

Full inputs: pred/target [524288, 3, 14] fp32. Output: [3] fp32.

Strategy (pure data parallel, 8 cores x 65536 samples):
  - Layout: 128 partitions x 512 samples/partition, samples innermost so the
    bulk fp16 elementwise work hits the DVE 2x mode. Inputs are converted
    once to persistent fp16 SBUF tiles (with a global 1/sqrt(8) prescale) and
    never re-streamed.
  - The tensor engine (PE) acts as a free accumulator: identity-weight
    matmuls into PSUM replace the j-sum trees (cross-covariance G, joint sums
    SP/ST) and the 3-way coordinate sums for the per-joint norms.
  - Kabsch/SVD is replaced by a closed form: K = H^T H, largest eigenvalue
    via cubic Newton (Cardano-bound start, 2 iters), remaining eigenvalues by
    quadratic deflation, lambda = s1+s2+sign(det H)*s3, then Markley's FOAM
    formula for R. Slab math fp16, eigen chain fp32.
  - Each core returns [128, 48] partial sums; host reduces in float64.
"""

import numpy as np

import concourse.bass as bass
import concourse.bacc as bacc
import concourse.mybir as mybir
import concourse.tile as tile
from concourse.bass_utils import run_bass_kernel_spmd
from concourse.masks import make_identity

F32 = mybir.dt.float32
F16 = mybir.dt.float16
AX = mybir.AluOpType
AF = mybir.ActivationFunctionType

N_CORES = 8
B_FULL = 524288
B_LOC = B_FULL // N_CORES          # 65536
P = 128                            # partitions
S = B_LOC // P                     # 512 samples per partition
NB = 64                            # samples per chunk (per partition)
NCHUNK = S // NB                   # 8
CJ = 42                            # 3*14
SF = 256                           # FOAM half size
SCALE = float(1.0 / np.sqrt(8.0))  # global input prescale (folded out on host)
SQ14I = float(1.0 / np.sqrt(14.0))
NACC = 2 * NCHUNK                  # accum slots per metric (2 PSUM subs/chunk)


def _pass1_chunk(nc, loadp, workp, pv, tv, p16, t16, Gp2, Gp1, SPp, STp,
                 n2M, n2A, accM, accA, G16, SPh, STh, I16, ci):
    cs = slice(ci * NB, (ci + 1) * NB)
    x32p = loadp.tile([P, NB, CJ], F32, tag="p32", name="x32p")
    nc.sync.dma_start(x32p[:], pv[:, cs, :])
    x32t = loadp.tile([P, NB, CJ], F32, tag="t32", name="x32t")
    nc.sync.dma_start(x32t[:], tv[:, cs, :])

    # fp32 -> fp16 J-major convert with the global prescale folded in.
    # On Pool: ACT is the pass-1 critical engine, Pool is idle here.
    nc.gpsimd.tensor_scalar_mul(
        p16[:, :, :, cs],
        x32p[:].rearrange("p s (c j) -> p c j s", c=3, j=14), SCALE)
    nc.gpsimd.tensor_scalar_mul(
        t16[:, :, :, cs],
        x32t[:].rearrange("p s (c j) -> p c j s", c=3, j=14), SCALE)

    pcs = p16[:, :, :, cs]
    tcs = t16[:, :, :, cs]

    # ---- mpjpe: d, d^2, PE c-sum, sqrt-accum --------------------------------
    d = workp.tile([P, 3, 14, NB], F16, tag="d", name="d")
    nc.vector.tensor_tensor(d[:], pcs, tcs, op=AX.subtract)
    d2 = workp.tile([P, 3, 14, NB], F16, tag="d2", name="d2")
    nc.scalar.square(d2[:], d[:])
    for sub in range(2):
        ss = slice(sub * 32, sub * 32 + 32)
        for c in range(3):
            nc.tensor.matmul(n2M[sub][:], I16[:], d2[:, c, :, ss],
                             start=(c == 0), stop=(c == 2))
        scrM = workp.tile([P, 14, 32], F16, tag="scrM", name="scrM")
        nc.scalar.activation(scrM[:], n2M[sub][:], AF.Sqrt,
                             accum_out=accM[:, 2 * ci + sub:2 * ci + sub + 1])

    # ---- accel: second difference over j, squares, PE c-sum ----------------
    ta = workp.tile([P, 3, 12, NB], F16, tag="ta", name="ta")
    nc.vector.tensor_scalar_mul(ta[:], pcs[:, :, 1:13, :], -2.0)
    nc.vector.tensor_tensor(ta[:], ta[:], pcs[:, :, 0:12, :], op=AX.add)
    nc.vector.tensor_tensor(ta[:], ta[:], pcs[:, :, 2:14, :], op=AX.add)
    a2 = workp.tile([P, 3, 12, NB], F16, tag="a2", name="a2")
    nc.scalar.square(a2[:], ta[:])
    for sub in range(2):
        ss = slice(sub * 32, sub * 32 + 32)
        for c in range(3):
            nc.tensor.matmul(n2A[sub][:], I16[:], a2[:, c, :, ss],
                             start=(c == 0), stop=(c == 2))
        scrA = workp.tile([P, 12, 32], F16, tag="scrA", name="scrA")
        nc.scalar.activation(scrA[:], n2A[sub][:], AF.Sqrt,
                             accum_out=accA[:, 2 * ci + sub:2 * ci + sub + 1])

    # ---- G / SP / ST via PE -------------------------------------------------
    # prod[k, i, j, s] = p_i t_k; one TT per k keeps APs within 3 free dims.
    CS = [P, 3, 14, NB]
    prod = workp.tile([P, 3, 3, 14, NB], F16, tag="prod", name="prod")
    for k in range(3):
        nc.vector.tensor_tensor(
            prod[:, k], pcs,
            tcs[:, k].unsqueeze(1).broadcast_to(CS), op=AX.mult)
    # G16[k, i] = sum_j prod[k, i, j]; split k to fit PSUM banks
    for (gp, ksl, nk) in ((Gp2, slice(0, 2), 2), (Gp1, slice(2, 3), 1)):
        for j in range(14):
            nc.tensor.matmul(gp[:], I16[:], prod[:, ksl, :, j, :],
                             start=(j == 0), stop=(j == 13))
    for j in range(14):
        nc.tensor.matmul(SPp[:], I16[:], p16[:, :, j, cs],
                         start=(j == 0), stop=(j == 13))
    for j in range(14):
        nc.tensor.matmul(STp[:], I16[:], t16[:, :, j, cs],
                         start=(j == 0), stop=(j == 13))

    # drains: G + SP/ST on ACT (GPSIMD cannot read PSUM)
    nc.scalar.copy(G16[:, 0:2, :, cs], Gp2[:])
    nc.scalar.copy(G16[:, 2:3, :, cs], Gp1[:])
    nc.scalar.activation(SPh[:, :, cs], SPp[:], AF.Copy, scale=SQ14I)
    nc.scalar.activation(STh[:, :, cs], STp[:], AF.Copy, scale=SQ14I)


def _foam_half_a(nc, sp_, chp, G16, SPh, STh, R16, V16, hf):
    """FOAM part A: H, K, invariants, eigen chain, adjugate, fp16 staging.

    H is in s^2 = 1/8 scale (inherited from the input prescale); the FOAM
    formula is scale-invariant so no rescaling is needed anywhere.
    SPh/STh are joint sums scaled by 1/sqrt(14).
    """
    fs = slice(hf * SF, hf * SF + SF)
    S3 = [P, 3, 3, SF]
    # G16 is stored (k, i); present it as (i, k) via a stride view
    Gv = G16[:, :, :, fs].transpose([0, 2, 1, 3])
    SPv = SPh[:, :, fs]
    STv = STh[:, :, fs]

    def slab(name):
        # rotating scratch slab; at most `bufs` of these live at once
        return sp_.tile(S3, F16, tag="ktmp", name=name)

    def ch(name, dt=F32):
        return chp.tile([P, SF], dt, tag="ch32" if dt == F32 else "ch16",
                        name=name)

    def named(tag, dt=F32):
        return chp.tile([P, SF], dt, tag=tag, name=tag, bufs=1)

    # H = G - SP ST^T / 14  (SPh*STh = SP*ST/14 already)
    outer = slab("outer")
    nc.vector.tensor_tensor(
        outer[:], SPv.unsqueeze(2).broadcast_to(S3),
        STv.unsqueeze(1).broadcast_to(S3), op=AX.mult)
    H16 = sp_.tile(S3, F16, tag="H16", bufs=1, name="H16")
    nc.vector.tensor_tensor(H16[:], Gv, outer[:], op=AX.subtract)

    # detH on Pool (fp32 out), from fp16 H
    detH = named("detH")
    c1 = ch("det_c1"); c2 = ch("det_c2"); acc = ch("det_acc")
    nc.gpsimd.tensor_tensor(c1[:], H16[:, 1, 1], H16[:, 2, 2], op=AX.mult)
    nc.gpsimd.tensor_tensor(c2[:], H16[:, 1, 2], H16[:, 2, 1], op=AX.mult)
    nc.gpsimd.tensor_tensor(c1[:], c1[:], c2[:], op=AX.subtract)
    nc.gpsimd.tensor_tensor(acc[:], H16[:, 0, 0], c1[:], op=AX.mult)
    nc.gpsimd.tensor_tensor(c1[:], H16[:, 1, 0], H16[:, 2, 2], op=AX.mult)
    nc.gpsimd.tensor_tensor(c2[:], H16[:, 1, 2], H16[:, 2, 0], op=AX.mult)
    nc.gpsimd.tensor_tensor(c1[:], c1[:], c2[:], op=AX.subtract)
    nc.gpsimd.tensor_tensor(c1[:], H16[:, 0, 1], c1[:], op=AX.mult)
    nc.gpsimd.tensor_tensor(acc[:], acc[:], c1[:], op=AX.subtract)
    nc.gpsimd.tensor_tensor(c1[:], H16[:, 1, 0], H16[:, 2, 1], op=AX.mult)
    nc.gpsimd.tensor_tensor(c2[:], H16[:, 1, 1], H16[:, 2, 0], op=AX.mult)
    nc.gpsimd.tensor_tensor(c1[:], c1[:], c2[:], op=AX.subtract)
    nc.gpsimd.tensor_tensor(c1[:], H16[:, 0, 2], c1[:], op=AX.mult)
    nc.gpsimd.tensor_tensor(detH[:], acc[:], c1[:], op=AX.add)

    # K = H^T H (fp16 slabs, accumulate into K16 with one rotating temp)
    K16 = sp_.tile(S3, F16, tag="K16", bufs=1, name="K16")
    nc.vector.tensor_tensor(K16[:], H16[:, 0].unsqueeze(2).broadcast_to(S3),
                            H16[:, 0].unsqueeze(1).broadcast_to(S3), op=AX.mult)
    for c in (1, 2):
        tc_ = slab(f"t{c}")
        nc.vector.tensor_tensor(tc_[:], H16[:, c].unsqueeze(2).broadcast_to(S3),
                                H16[:, c].unsqueeze(1).broadcast_to(S3),
                                op=AX.mult)
        nc.vector.tensor_tensor(K16[:], K16[:], tc_[:], op=AX.add)

    # invariants: m2 = tr K (fp32), I3 = detH^2, I2 via Pool
    m2 = named("m2")
    nc.vector.tensor_tensor(m2[:], K16[:, 0, 0], K16[:, 1, 1], op=AX.add)
    nc.vector.tensor_tensor(m2[:], m2[:], K16[:, 2, 2], op=AX.add)
    I3 = named("I3")
    nc.vector.tensor_tensor(I3[:], detH[:], detH[:], op=AX.mult)

    o01 = ch("o01"); o02 = ch("o02"); o12 = ch("o12")
    nc.scalar.square(o01[:], K16[:, 0, 1])
    nc.scalar.square(o02[:], K16[:, 0, 2])
    nc.scalar.square(o12[:], K16[:, 1, 2])
    I2 = named("I2"); mm = ch("mm")
    nc.gpsimd.tensor_tensor(I2[:], K16[:, 0, 0], K16[:, 1, 1], op=AX.mult)
    nc.gpsimd.tensor_tensor(I2[:], I2[:], o01[:], op=AX.subtract)
    nc.gpsimd.tensor_tensor(mm[:], K16[:, 0, 0], K16[:, 2, 2], op=AX.mult)
    nc.gpsimd.tensor_tensor(mm[:], mm[:], o02[:], op=AX.subtract)
    nc.gpsimd.tensor_tensor(I2[:], I2[:], mm[:], op=AX.add)
    nc.gpsimd.tensor_tensor(mm[:], K16[:, 1, 1], K16[:, 2, 2], op=AX.mult)
    nc.gpsimd.tensor_tensor(mm[:], mm[:], o12[:], op=AX.subtract)
    nc.gpsimd.tensor_tensor(I2[:], I2[:], mm[:], op=AX.add)

    # Cardano upper bound start: x0 = m2/3 + 2*sqrt((dsum + 2*osum)/6)
    q = named("q")
    nc.vector.tensor_scalar_mul(q[:], m2[:], 1.0 / 3.0)
    osum = ch("osum")
    nc.vector.tensor_tensor(osum[:], o01[:], o02[:], op=AX.add)
    nc.vector.tensor_tensor(osum[:], osum[:], o12[:], op=AX.add)
    dsum = ch("dsum"); kd = ch("kd"); kd2 = ch("kd2")
    nc.vector.tensor_tensor(kd[:], K16[:, 0, 0], q[:], op=AX.subtract)
    nc.vector.tensor_tensor(dsum[:], kd[:], kd[:], op=AX.mult)
    nc.vector.tensor_tensor(kd[:], K16[:, 1, 1], q[:], op=AX.subtract)
    nc.vector.tensor_tensor(kd2[:], kd[:], kd[:], op=AX.mult)
    nc.vector.tensor_tensor(dsum[:], dsum[:], kd2[:], op=AX.add)
    nc.vector.tensor_tensor(kd[:], K16[:, 2, 2], q[:], op=AX.subtract)
    nc.vector.tensor_tensor(kd2[:], kd[:], kd[:], op=AX.mult)
    nc.vector.tensor_tensor(dsum[:], dsum[:], kd2[:], op=AX.add)
    p2 = ch("p2")
    nc.vector.scalar_tensor_tensor(p2[:], osum[:], 2.0, dsum[:],
                                   op0=AX.mult, op1=AX.add)
    pC = ch("pC")
    nc.scalar.activation(pC[:], p2[:], AF.Sqrt, scale=1.0 / 6.0)
    X = named("X")
    nc.vector.scalar_tensor_tensor(X[:], pC[:], 2.0, q[:],
                                   op0=AX.mult, op1=AX.add)

    # Newton on f(x) = ((x - m2) x + I2) x - I3, 2 iters from above
    m2_2 = named("m2_2")
    nc.vector.tensor_scalar_mul(m2_2[:], m2[:], 2.0)
    na = ch("na"); nb = ch("nb")
    for _ in range(2):
        nc.vector.tensor_tensor(na[:], X[:], m2[:], op=AX.subtract)
        nc.vector.tensor_tensor(na[:], na[:], X[:], op=AX.mult)
        nc.vector.tensor_tensor(na[:], na[:], I2[:], op=AX.add)
        nc.vector.tensor_tensor(na[:], na[:], X[:], op=AX.mult)
        nc.vector.tensor_tensor(na[:], na[:], I3[:], op=AX.subtract)   # f
        nc.vector.tensor_scalar_mul(nb[:], X[:], 3.0)
        nc.vector.tensor_tensor(nb[:], nb[:], m2_2[:], op=AX.subtract)
        nc.vector.tensor_tensor(nb[:], nb[:], X[:], op=AX.mult)
        nc.vector.tensor_tensor(nb[:], nb[:], I2[:], op=AX.add)        # f'
        nc.vector.reciprocal(nb[:], nb[:])
        nc.vector.tensor_tensor(na[:], na[:], nb[:], op=AX.mult)
        nc.vector.tensor_tensor(X[:], X[:], na[:], op=AX.subtract)

    # deflate: mu2/mu3 from x^2 - (m2-mu1)x + I3/mu1
    mus = chp.tile([P, 3, SF], F32, tag="mus", name="mus", bufs=1)
    mu1 = mus[:, 0]; mu2 = mus[:, 1]; mu3 = mus[:, 2]
    nc.vector.tensor_scalar_max(mu1, X[:], 1e-25)
    b = ch("b"); cc = ch("cc"); rmu = ch("rmu")
    nc.vector.tensor_tensor(b[:], m2[:], mu1, op=AX.subtract)
    nc.vector.reciprocal(rmu[:], mu1)
    nc.vector.tensor_tensor(cc[:], I3[:], rmu[:], op=AX.mult)
    b2 = ch("b2")
    nc.vector.tensor_tensor(b2[:], b[:], b[:], op=AX.mult)
    disc2 = ch("disc2")
    nc.vector.scalar_tensor_tensor(disc2[:], cc[:], -4.0, b2[:],
                                   op0=AX.mult, op1=AX.add)
    nc.vector.tensor_scalar_max(disc2[:], disc2[:], 0.0)
    disc = ch("disc")
    nc.scalar.sqrt(disc[:], disc2[:])
    bh = ch("bh")
    nc.vector.tensor_scalar_mul(bh[:], b[:], 0.5)
    nc.vector.scalar_tensor_tensor(mu2, disc[:], 0.5, bh[:],
                                   op0=AX.mult, op1=AX.add)
    nc.vector.tensor_scalar_max(mu2, mu2, 0.0)
    nc.vector.tensor_tensor(mu3, b[:], mu2, op=AX.subtract)
    nc.vector.tensor_scalar_max(mu3, mu3, 0.0)

    rt = chp.tile([P, 3, SF], F32, tag="rt", name="rt", bufs=1)
    nc.scalar.sqrt(rt[:], mus[:])
    sgn = ch("sgn")
    nc.scalar.sign(sgn[:], detH[:])
    lam = named("lam")
    nc.vector.tensor_tensor(lam[:], rt[:, 0], rt[:, 1], op=AX.add)
    s3s = ch("s3s")
    nc.vector.tensor_tensor(s3s[:], sgn[:], rt[:, 2], op=AX.mult)
    nc.vector.tensor_tensor(lam[:], lam[:], s3s[:], op=AX.add)

    # alpha2 = lam^2 + m2 ; zeta2 = (lam^2 - m2) lam - 2 detH (floored)
    lam2 = ch("lam2"); alpha2 = named("alpha2")
    nc.vector.tensor_tensor(lam2[:], lam[:], lam[:], op=AX.mult)
    nc.vector.tensor_tensor(alpha2[:], lam2[:], m2[:], op=AX.add)
    zt = ch("zt")
    nc.vector.tensor_tensor(zt[:], lam2[:], m2[:], op=AX.subtract)
    nc.vector.tensor_tensor(zt[:], zt[:], lam[:], op=AX.mult)
    zeta2 = ch("zeta2")
    nc.vector.scalar_tensor_tensor(zeta2[:], detH[:], -2.0, zt[:],
                                   op0=AX.mult, op1=AX.add)
    m2s = ch("m2s")
    nc.scalar.sqrt(m2s[:], m2[:])
    zfl = ch("zfl")
    nc.vector.scalar_tensor_tensor(zfl[:], m2s[:], 1e-4, m2[:],
                                   op0=AX.mult, op1=AX.mult)
    nc.vector.tensor_tensor(zeta2[:], zeta2[:], zfl[:], op=AX.max)
    rz = ch("rz")
    nc.vector.reciprocal(rz[:], zeta2[:])

    # fp16 stage for the slab assembly
    a16 = named("a16", F16)
    nc.vector.tensor_copy(a16[:], alpha2[:])
    l16 = named("l16", F16)
    nc.vector.tensor_scalar_mul(l16[:], lam[:], 2.0)
    rz16 = named("rz16", F16)
    nc.vector.tensor_copy(rz16[:], rz[:])

    # adjugate of H: fp16 channel ops on DVE (cheap in 2x mode)
    adjH = sp_.tile(S3, F16, tag="adjH", bufs=1, name="adjH")
    idx = [
        (0, 0, (1, 1), (2, 2), (1, 2), (2, 1)),
        (0, 1, (0, 2), (2, 1), (0, 1), (2, 2)),
        (0, 2, (0, 1), (1, 2), (0, 2), (1, 1)),
        (1, 0, (1, 2), (2, 0), (1, 0), (2, 2)),
        (1, 1, (0, 0), (2, 2), (0, 2), (2, 0)),
        (1, 2, (0, 2), (1, 0), (0, 0), (1, 2)),
        (2, 0, (1, 0), (2, 1), (1, 1), (2, 0)),
        (2, 1, (0, 1), (2, 0), (0, 0), (2, 1)),
        (2, 2, (0, 0), (1, 1), (0, 1), (1, 0)),
    ]
    aw1 = ch("aw1", F16); aw2 = ch("aw2", F16)
    for (i, j, (a1, a2), (b1, b2), (c1_, c2_), (d1, d2)) in idx:
        nc.vector.tensor_tensor(aw1[:], H16[:, a1, a2], H16[:, b1, b2], op=AX.mult)
        nc.vector.tensor_tensor(aw2[:], H16[:, c1_, c2_], H16[:, d1, d2], op=AX.mult)
        nc.vector.tensor_tensor(adjH[:, i, j], aw1[:], aw2[:], op=AX.subtract)

    return {"H16": H16, "K16": K16, "adjH": adjH,
            "a16": a16, "l16": l16, "rz16": rz16}


def _foam_half_b(nc, sp_, chp, st, G16, SPh, STh, R16, V16, t16, hf):
    """FOAM part B: slab assembly, R, V, and the V fold into t16."""
    fs = slice(hf * SF, hf * SF + SF)
    S3 = [P, 3, 3, SF]
    SPv = SPh[:, :, fs]
    STv = STh[:, :, fs]
    H16 = st["H16"]; K16 = st["K16"]; adjH = st["adjH"]
    a16 = st["a16"]; l16 = st["l16"]; rz16 = st["rz16"]

    def slab(name):
        return sp_.tile(S3, F16, tag="ktmp", name=name)

    # num = (alpha2 I - 2K) H^T + 2 lam adjH ;  R = num / zeta2, clamped
    W = sp_.tile(S3, F16, tag="Mt", bufs=1, name="W")
    nc.vector.tensor_scalar_mul(W[:], K16[:], -2.0)
    # diagonal view: stride 4*SF within the contiguous [3,3,SF] block
    nc.vector.tensor_tensor(
        W[:].rearrange("p a b s -> p (a b) s")[:, 0:9:4, :],
        W[:].rearrange("p a b s -> p (a b) s")[:, 0:9:4, :],
        a16[:].unsqueeze(1).broadcast_to([P, 3, SF]), op=AX.add)
    Ht = H16[:].transpose([0, 2, 1, 3])
    num = slab("num")
    nc.vector.tensor_tensor(num[:], W[:, :, 0].unsqueeze(2).broadcast_to(S3),
                            H16[:, :, 0].unsqueeze(1).broadcast_to(S3), op=AX.mult)
    for c in (1, 2):
        uc = slab(f"u{c}")
        nc.vector.tensor_tensor(uc[:], W[:, :, c].unsqueeze(2).broadcast_to(S3),
                                H16[:, :, c].unsqueeze(1).broadcast_to(S3),
                                op=AX.mult)
        nc.vector.tensor_tensor(num[:], num[:], uc[:], op=AX.add)
    vB = slab("vB")
    nc.vector.tensor_tensor(
        vB[:], l16[:].unsqueeze(1).unsqueeze(2).broadcast_to(S3), adjH[:],
        op=AX.mult)
    nc.vector.tensor_tensor(num[:], num[:], vB[:], op=AX.add)
    R16v = R16[:, :, :, fs]
    nc.vector.tensor_tensor(
        R16v, num[:], rz16[:].unsqueeze(1).unsqueeze(2).broadcast_to(S3),
        op=AX.mult)
    nc.vector.tensor_scalar(R16v, R16v, 4.0, -4.0, op0=AX.min, op1=AX.max)

    # V = (STh - R SPh) / sqrt(14)  (== t_mean - R p_mean)
    pv_ = slab("pv_")
    nc.vector.tensor_tensor(pv_[:], R16v, SPv.unsqueeze(1).broadcast_to(S3),
                            op=AX.mult)
    RS = chp.tile([P, 3, SF], F16, tag="RS", name="RS", bufs=1)
    nc.vector.tensor_tensor(RS[:], pv_[:, :, 0], pv_[:, :, 1], op=AX.add)
    nc.vector.tensor_tensor(RS[:], RS[:], pv_[:, :, 2], op=AX.add)
    Vt = chp.tile([P, 3, SF], F16, tag="Vt", name="Vt", bufs=1)
    nc.vector.tensor_tensor(Vt[:], STv, RS[:], op=AX.subtract)
    nc.vector.tensor_scalar_mul(V16[:, :, fs], Vt[:], SQ14I)

    # fold V into t16 in place: pass3's residual becomes qv - t16
    TSH = [P, 3, 14, SF]
    nc.vector.tensor_tensor(
        t16[:, :, :, fs], t16[:, :, :, fs],
        V16[:, :, fs].unsqueeze(2).broadcast_to(TSH), op=AX.subtract)


def _pass3_chunk(nc, workp, p16, t16, R16, V16, n2P, dvps, accP, I16, nI16, ci):
    cs = slice(ci * NB, (ci + 1) * NB)
    CS = [P, 3, 14, NB]
    # prq[k][i, j, s] = R_ik p_kj ; PE sums over k and subtracts t16 (V folded)
    prqs = []
    for k in range(3):
        prq = workp.tile(CS, F16, tag=f"prq{k}", name=f"prq{k}")
        nc.vector.tensor_tensor(
            prq[:], R16[:, :, k, cs].unsqueeze(2).broadcast_to(CS),
            p16[:, k, :, cs].unsqueeze(1).broadcast_to(CS), op=AX.mult)
        prqs.append(prq)
    dv2 = workp.tile(CS, F16, tag="dv2", name="dv2")
    tcs = t16[:, :, :, cs]
    subs = [(i * 12, min(12, NB - i * 12)) for i in range((NB + 11) // 12)]
    for si, (s0, sw) in enumerate(subs):
        ss = slice(s0, s0 + sw)
        dvp = dvps[si % len(dvps)]
        for k in range(3):
            nc.tensor.matmul(dvp[:, :, :, 0:sw], I16[:], prqs[k][:, :, :, ss],
                             start=(k == 0), stop=False)
        nc.tensor.matmul(dvp[:, :, :, 0:sw], nI16[:], tcs[:, :, :, ss],
                         start=False, stop=True)
        nc.scalar.square(dv2[:, :, :, ss], dvp[:, :, :, 0:sw])
    for sub in range(2):
        ss = slice(sub * 32, sub * 32 + 32)
        for c in range(3):
            nc.tensor.matmul(n2P[sub][:], I16[:], dv2[:, c, :, ss],
                             start=(c == 0), stop=(c == 2))
        scrP = workp.tile([P, 14, 32], F16, tag="scrP", name="scrP")
        nc.scalar.activation(scrP[:], n2P[sub][:], AF.Sqrt,
                             accum_out=accP[:, 2 * ci + sub:2 * ci + sub + 1])


def build_bass():
    nc = bacc.Bacc("TRN2")
    pred = nc.dram_tensor("pred", [B_LOC, CJ], F32, kind="ExternalInput")
    targ = nc.dram_tensor("target", [B_LOC, CJ], F32, kind="ExternalInput")
    out = nc.dram_tensor("out", [P, 3 * NACC], F32, kind="ExternalOutput")

    pv = pred[:].rearrange("(p n) d -> p n d", p=P)   # [128, 512, 42]
    tv = targ[:].rearrange("(p n) d -> p n d", p=P)

    with tile.TileContext(nc) as tc:
        with tc.tile_pool(name="persist", bufs=1) as pp:
            p16 = pp.tile([P, 3, 14, S], F16, tag="p16")
            t16 = pp.tile([P, 3, 14, S], F16, tag="t16")
            G16 = pp.tile([P, 3, 3, S], F16, tag="G16")
            SPh = pp.tile([P, 3, S], F16, tag="SPh")
            STh = pp.tile([P, 3, S], F16, tag="STh")
            R16 = pp.tile([P, 3, 3, S], F16, tag="R16")
            V16 = pp.tile([P, 3, S], F16, tag="V16")
            accM = pp.tile([P, NACC], F32, tag="accM")
            accA = pp.tile([P, NACC], F32, tag="accA")
            accP = pp.tile([P, NACC], F32, tag="accP")
            I16 = pp.tile([P, P], F16, tag="I16")
            make_identity(nc, I16[:])
            nI16 = pp.tile([P, P], F16, tag="nI16")
            nc.vector.tensor_scalar_mul(nI16[:], I16[:], -1.0)

            # ---------------- pass 1 ----------------------------------------
            with tc.tile_pool(name="load1", bufs=2) as loadp, \
                 tc.tile_pool(name="work1", bufs=1) as workp, \
                 tc.tile_pool(name="ps1", bufs=1, space="PSUM") as psp:
                Gp2 = psp.tile([P, 2, 3, NB], F32, tag="Gp2")
                Gp1 = psp.tile([P, 1, 3, NB], F32, tag="Gp1")
                SPp = psp.tile([P, 3, NB], F32, tag="SPp")
                STp = psp.tile([P, 3, NB], F32, tag="STp")
                n2M = [psp.tile([P, 14, 32], F32, tag=f"n2M{s}", name=f"n2M{s}")
                       for s in range(2)]
                n2A = [psp.tile([P, 12, 32], F32, tag=f"n2A{s}", name=f"n2A{s}")
                       for s in range(2)]
                for ci in range(NCHUNK):
                    _pass1_chunk(nc, loadp, workp, pv, tv, p16, t16,
                                 Gp2, Gp1, SPp, STp, n2M, n2A,
                                 accM, accA, G16, SPh, STh, I16, ci)

            # ---------------- FOAM + pass 3, interleaved --------------------
            with tc.tile_pool(name="slab_a", bufs=2) as sp_a, \
                 tc.tile_pool(name="ch_a", bufs=14) as chp_a, \
                 tc.tile_pool(name="work3", bufs=1) as workp3, \
                 tc.tile_pool(name="ps3", bufs=1, space="PSUM") as psp3:
                n2P = [psp3.tile([P, 14, 32], F32, tag=f"n2P{s}", name=f"n2P{s}")
                       for s in range(2)]
                dvps = [psp3.tile([P, 3, 14, 12], F32, tag=f"dvp{s}",
                                  name=f"dvp{s}") for s in range(2)]
                st0 = _foam_half_a(nc, sp_a, chp_a, G16, SPh, STh, R16, V16, 0)
                _foam_half_b(nc, sp_a, chp_a, st0, G16, SPh, STh, R16, V16,
                             t16, 0)
                st1 = _foam_half_a(nc, sp_a, chp_a, G16, SPh, STh, R16, V16, 1)
                for ci in range(4):
                    _pass3_chunk(nc, workp3, p16, t16, R16, V16, n2P, dvps,
                                 accP, I16, nI16, ci)
                _foam_half_b(nc, sp_a, chp_a, st1, G16, SPh, STh, R16, V16,
                             t16, 1)
                for ci in range(4, NCHUNK):
                    _pass3_chunk(nc, workp3, p16, t16, R16, V16, n2P, dvps,
                                 accP, I16, nI16, ci)

            stage = pp.tile([P, 3 * NACC], F32, tag="stage", name="stage")
            nc.gpsimd.tensor_copy(stage[:, 0:NACC], accM[:])
            nc.gpsimd.tensor_copy(stage[:, NACC:2 * NACC], accP[:])
            nc.gpsimd.tensor_copy(stage[:, 2 * NACC:3 * NACC], accA[:])
            nc.sync.dma_start(out[:], stage[:])

    nc.finalize()
    return nc


_NC = None


def kernel(pred: np.ndarray, target: np.ndarray) -> np.ndarray:
    global _NC
    if _NC is None:
        _NC = build_bass()

    pred = np.ascontiguousarray(pred, dtype=np.float32).reshape(B_FULL, CJ)
    target = np.ascontiguousarray(target, dtype=np.float32).reshape(B_FULL, CJ)

    in_maps = []
    for c in range(N_CORES):
        sl = slice(c * B_LOC, (c + 1) * B_LOC)
        in_maps.append({"pred": pred[sl], "target": target[sl]})

    res = run_bass_kernel_spmd(_NC, in_maps, core_ids=list(range(N_CORES)))
    mp = pa = ac = 0.0
    for r in res.results:
        o = r["out"].astype(np.float64)
        mp += o[:, 0:NACC].sum()
        pa += o[:, NACC:2 * NACC].sum()
        ac += o[:, 2 * NACC:3 * NACC].sum()
    inv = 1.0 / SCALE
    return np.array([mp / (B_FULL * 14) * inv,
                     pa / (B_FULL * 14) * inv,
                     ac / (B_FULL * 12) * inv], dtype=np.float32)
